# revision 1
# baseline (speedup 1.0000x reference)
"""Trainium2 Bass kernel for a 2-layer GNN (message passing + MLP + global mean pool).

Reference computation (per graph batch):
    mp(h)[r] = 2*h[r] + sum_{e: row[e]==r} h[col[e]]      (self loop + residual fold)
    h1 = relu(mp(x) @ W1 + b1)
    h2 = relu(mp(h1) @ W2 + b2)
    out = segment_mean(h2, batch) @ Wout + bout

Strategy (8 NeuronCores):
  - Destination-shard nodes: core c owns rows [c*S, (c+1)*S), S = N/8.
  - Host (index-only preprocessing): bucket edges by dest shard, sort by dest
    row-tile (128 rows), split by source parity (L1, packed x pair table) /
    source half (L2, int16 gather index range), pad chunk counts to the max
    across cores so all 8 cores run one program.
  - Device: dma_gather fetches bf16 source rows per edge, one call per block
    of B destination tiles; scatter-add runs on the TensorEngine as one-hot
    matmuls (P[k,r] = (dst[k]==r)) accumulating in PSUM per 128-row tile.
    P is built with one DVE is_equal per tile using broadcast access patterns.
  - Between layers: AllGather of the bf16 h1 shards (each core needs random
    remote rows in layer 2).
  - Global mean pool via the same one-hot-matmul trick against graph ids,
    partial sums AllReduced across cores, final tiny matmul on every core.
"""

import os
import sys

for _p in ("/opt/trn_rl_repo", "/opt/pypackages"):
    if _p not in sys.path and os.path.isdir(_p):
        sys.path.append(_p)

import numpy as np
import ml_dtypes

BF16 = ml_dtypes.bfloat16

# Problem constants (nn_BasicGNN: N=50000 nodes, E=800000 edges).
N, E, IN, H, OUT, G = 50000, 800000, 64, 128, 10, 64
C = 8              # cores
S = N // C         # 6250 rows per shard
TP = 128           # rows per destination tile
T = (S + TP - 1) // TP   # 49 tiles per shard
SP = T * TP        # padded shard rows (6272)
NH = N // 2        # 25000: x pair-table rows
TA = (T + 1) // 2  # tiles in AllGather chunk A (chunk B = rest)
RA = min(S, TA * TP)   # rows per core in chunk A
RB = S - RA            # rows per core in chunk B
B = 4              # destination tiles per gather block

PAD_DST = 255.0    # dest offset for padding messages (no row matches -> adds 0)

LAST_EXEC_NS = None
LAST_RESULTS = None


def _set_dims(n=None, e=None, g=None):
    """Override problem dims (for small-scale simulation tests only)."""
    global N, E, G, S, T, SP, NH
    if n is not None:
        N = n
    if e is not None:
        E = e
    if g is not None:
        G = g
    global TA, RA, RB
    S = N // C
    T = (S + TP - 1) // TP
    SP = T * TP
    NH = N // 2
    TA = (T + 1) // 2
    RA = min(S, TA * TP)
    RB = S - RA


def _blocks():
    return [(b, min(b + B, T)) for b in range(0, T, B)]


def _wrap_idx(a):
    """int16 index array [K] (K%16==0) -> [128, K//16] in dma_gather layout:
    index i lives at [i % 16, i // 16], replicated for the 8 gpsimd cores."""
    K = a.shape[0]
    w = a.reshape(K // 16, 16).T.astype(np.int16)
    return np.tile(w, (8, 1))


def _wrap_dst(d):
    """dest-offset array [M*128] -> [128, M] bf16; msg (c*128+k) -> [k, c]."""
    M = d.shape[0] // 128
    return d.reshape(M, 128).T.astype(np.float32)


class Plan:
    """Compile-time loop structure shared by all 8 cores + per-core tensors."""
    pass


def preprocess(x, edge_index, batch):
    """Index-only host preprocessing: edge bucketing/sorting + table packing."""
    plan = Plan()

    row = edge_index[0].astype(np.int64)
    col = edge_index[1].astype(np.int64)
    shard = row // S

    # counts per (core, tile, group) for both layers
    # L1 groups: source parity (pair table slicing); L2 groups: source half
    per_core = []
    cnt1 = np.zeros((C, T, 2), np.int64)
    cnt2 = np.zeros((C, T, 2), np.int64)
    for c in range(C):
        m = shard == c
        r = row[m] - c * S
        s = col[m]
        t = r // TP
        d = (r % TP).astype(np.float64)

        g1 = (s & 1).astype(np.int64)
        key1 = t * 2 + g1
        o1 = np.argsort(key1, kind="stable")
        cnt1[c] = np.bincount(key1, minlength=T * 2).reshape(T, 2)

        # L2: source split by local row range (matches the two tile-aligned
        # AllGather chunks); gather index into the rank-major chunk tables
        sr = s // S
        sl = s % S
        g2 = (sl >= RA).astype(np.int64)
        idx2v = np.where(g2 == 0, sr * RA + sl, sr * RB + (sl - RA))
        key2 = t * 2 + g2
        o2 = np.argsort(key2, kind="stable")
        cnt2[c] = np.bincount(key2, minlength=T * 2).reshape(T, 2)

        per_core.append(
            dict(
                idx1=(s >> 1)[o1], dst1=d[o1],
                idx2=idx2v[o2], dst2=d[o2],
            )
        )

    # chunk counts (of 128 messages), maxed across cores -> single program
    K1 = np.maximum(-(-cnt1 // 128), 0).max(axis=0)   # [T, 2]
    K2 = np.maximum(-(-cnt2 // 128), 0).max(axis=0)   # [T, 2]
    plan.K1 = K1
    plan.K2 = K2
    plan.M1 = K1.sum(axis=1)      # chunks per tile, layer 1
    plan.M2 = K2.sum(axis=1)      # chunks per tile, layer 2

    def starts_of(cnt):
        starts = np.zeros((T, 2), np.int64)
        p = 0
        for t in range(T):
            for g in range(2):
                starts[t, g] = p
                p += cnt[t, g]
        return starts

    def grab(idx, dst, starts, cnt, K, t, g):
        n = int(cnt[t, g])
        k = int(K[t, g])
        s0 = int(starts[t, g])
        ii = idx[s0:s0 + n]
        dd = dst[s0:s0 + n]
        pad = k * 128 - n
        if pad:
            ii = np.concatenate([ii, np.zeros(pad, np.int64)])
            dd = np.concatenate([dd, np.full(pad, PAD_DST)])
        return ii, dd

    def pack_l1(idx, dst, cnt):
        """L1 flat order: per tile [parity0 pad][parity1 pad]."""
        starts = starts_of(cnt)
        idx_out, dst_out = [], []
        for t in range(T):
            for g in range(2):
                ii, dd = grab(idx, dst, starts, cnt, K1, t, g)
                idx_out.append(ii)
                dst_out.append(dd)
        return np.concatenate(idx_out), np.concatenate(dst_out)

    def pack_l2(idx, dst, cnt):
        """L2 flat order: per B-tile block [t0 lo..t3 lo][t0 hi..t3 hi]."""
        starts = starts_of(cnt)
        idx_out, dst_out = [], []
        for b0, b1 in _blocks():
            for g in range(2):
                for t in range(b0, b1):
                    ii, dd = grab(idx, dst, starts, cnt, K2, t, g)
                    idx_out.append(ii)
                    dst_out.append(dd)
        return np.concatenate(idx_out), np.concatenate(dst_out)

    plan.idx1, plan.d1, plan.idx2, plan.d2 = [], [], [], []
    for c in range(C):
        pc = per_core[c]
        i1, dd1 = pack_l1(pc["idx1"], pc["dst1"], cnt1[c])
        i2, dd2 = pack_l2(pc["idx2"], pc["dst2"], cnt2[c])
        plan.idx1.append(_wrap_idx(i1))
        plan.d1.append(_wrap_dst(dd1))
        plan.idx2.append(_wrap_idx(i2))
        plan.d2.append(_wrap_dst(dd2))

    # per-core transposed x shard [IN, SP] (padded with zeros)
    plan.xT = []
    plan.xrow = []
    plan.batchf = []
    for c in range(C):
        xs = np.zeros((SP, IN), np.float32)
        xs[:S] = x[c * S:(c + 1) * S]
        plan.xT.append(np.ascontiguousarray(xs.T))
        # row-major per-tile layout [128, T*IN]: [p, t*IN+f] = x[t*128+p, f]
        plan.xrow.append(np.ascontiguousarray(
            xs.reshape(T, TP, IN).transpose(1, 0, 2).reshape(TP, T * IN)
            .astype(BF16)))
        bf = np.full(SP, float(G), np.float32)
        bf[:S] = batch[c * S:(c + 1) * S].astype(np.float32)
        plan.batchf.append(np.ascontiguousarray(bf.reshape(T, TP).T))  # [128,T]

    # graph counts -> reciprocal (index-derived)
    cnts = np.bincount(batch.astype(np.int64), minlength=G).astype(np.float32)
    plan.inv = (1.0 / np.maximum(cnts, 1.0)).astype(np.float32)
    return plan


def build_program(plan):
    import concourse.bacc as bacc
    import concourse.tile as tile
    import concourse.mybir as mybir
    import concourse.tile_sem_assignment as _tsa

    # SWDGE completion sems are HW-locked to the queue that first bumps them,
    # but TileClockTick rotates DMASW lanes queue-blind. Pin lane = queue_num
    # for multi-queue SWDGE ops (same-queue ops serialize on the ring anyway,
    # so sharing one lane per queue adds no false dependencies).
    if not getattr(_tsa.TileClockTick, "_gnn_queue_lanes", False):
        _orig_assign = _tsa.TileClockTick._assign_tick

        def _assign(self, inst):
            qn = getattr(inst, "queue_num", None)
            if (qn is not None
                    and inst.engine == _tsa.mybir.EngineType.Pool
                    and isinstance(inst, _tsa.DMAInst)):
                self.next_sw_dma_idx = int(qn)
            return _orig_assign(self, inst)

        _tsa.TileClockTick._assign_tick = _assign
        _tsa.TileClockTick._gnn_queue_lanes = True

    dt = mybir.dt
    f32, bf16, i16 = dt.float32, dt.bfloat16, dt.int16
    Alu = mybir.AluOpType
    Act = mybir.ActivationFunctionType

    phase = int(os.environ.get("GNN_PHASE", "4"))
    fakecoll = bool(int(os.environ.get("GNN_FAKECOLL", "0")))
    resfold = bool(int(os.environ.get("GNN_RESFOLD", "1")))

    M1, M2, K1, K2 = plan.M1, plan.M2, plan.K1, plan.K2
    SM1 = int(M1.sum())
    SM2 = int(M2.sum())
    L1W = SM1 * 8
    L2W = SM2 * 8

    # per-tile chunk base offsets (L1 flat order)
    off1 = np.concatenate([[0], np.cumsum(M1)]).astype(np.int64)
    blocks = _blocks()
    # L1 block bookkeeping: (chunk base, chunks in block)
    blk1 = [(int(off1[b0]), int(off1[b1] - off1[b0])) for b0, b1 in blocks]
    # L2 block bookkeeping: (chunk base, lo chunks, hi chunks, per-tile (klo,khi))
    blk2 = []
    cb = 0
    for b0, b1 in blocks:
        lo = int(K2[b0:b1, 0].sum())
        hi = int(K2[b0:b1, 1].sum())
        blk2.append((cb, lo, hi,
                     [(int(K2[t, 0]), int(K2[t, 1])) for t in range(b0, b1)]))
        cb += lo + hi

    nq = int(os.environ.get("GNN_NQ", "4"))
    nc = bacc.Bacc("TRN2", target_bir_lowering=False, debug=False,
                   num_devices=C, num_swdge_queues=nq)

    # ---- I/O -------------------------------------------------------------
    x_pair = nc.dram_tensor("x_pair", [NH, 2 * IN], bf16, kind="ExternalInput")
    xT_d = nc.dram_tensor("xT", [IN, SP], f32, kind="ExternalInput")
    xrow_d = nc.dram_tensor("xrow", [TP, T * IN], bf16, kind="ExternalInput")
    ident2_d = nc.dram_tensor("ident2", [TP, TP], bf16, kind="ExternalInput")
    W1e_d = nc.dram_tensor("W1e", [IN + 1, H], f32, kind="ExternalInput")
    W2_d = nc.dram_tensor("W2", [H, H], f32, kind="ExternalInput")
    b2b_d = nc.dram_tensor("b2b", [TP, H], f32, kind="ExternalInput")
    Wout_d = nc.dram_tensor("Wout", [H, OUT], f32, kind="ExternalInput")
    boutb_d = nc.dram_tensor("boutb", [G, OUT], f32, kind="ExternalInput")
    invb_d = nc.dram_tensor("invb", [TP, G], f32, kind="ExternalInput")
    iota128_d = nc.dram_tensor("iota128", [TP, TP], bf16, kind="ExternalInput")
    iota64_d = nc.dram_tensor("iota64", [TP, G], f32, kind="ExternalInput")
    batchf_d = nc.dram_tensor("batchf", [TP, T], f32, kind="ExternalInput")
    idx1_d = nc.dram_tensor("idx1", [TP, L1W], i16, kind="ExternalInput")
    d1_d = nc.dram_tensor("d1", [TP, SM1], f32, kind="ExternalInput")
    idx2_d = nc.dram_tensor("idx2", [TP, L2W], i16, kind="ExternalInput")
    d2_d = nc.dram_tensor("d2", [TP, SM2], f32, kind="ExternalInput")
    out_d = nc.dram_tensor("out", [G, OUT], f32, kind="ExternalOutput")

    with tile.TileContext(nc) as tc:
        from contextlib import ExitStack
        with ExitStack() as ctx:
            const = ctx.enter_context(tc.tile_pool(name="const", bufs=1))
            work = ctx.enter_context(tc.tile_pool(name="work", bufs=3))
            mpool = ctx.enter_context(tc.tile_pool(name="mpool", bufs=2))
            m1pool = ctx.enter_context(tc.tile_pool(name="m1pool", bufs=2))
            ppool = ctx.enter_context(tc.tile_pool(name="ppool", bufs=2))
            ipool = ctx.enter_context(tc.tile_pool(name="ipool", bufs=2))
            psum2 = ctx.enter_context(
                tc.tile_pool(name="psum2", bufs=2, space="PSUM"))
            psum1 = ctx.enter_context(
                tc.tile_pool(name="psum1", bufs=1, space="PSUM"))
            dram = ctx.enter_context(
                tc.tile_pool(name="dram", bufs=1, space="DRAM"))

            # ---- constants / persistent SBUF ----------------------------
            def load_const(dram_t, shape, dtype, tag):
                t = const.tile(shape, dtype, tag=tag)
                nc.sync.dma_start(t[:], dram_t[:, :])
                return t

            W1e_sb = load_const(W1e_d, [IN + 1, H], f32, "c_w1e")
            iota128_sb = load_const(iota128_d, [TP, TP], bf16, "c_iota128")
            pre_b = []
            W2_sb = load_const(W2_d, [H, H], f32, "c_w2")
            b2b_sb = load_const(b2b_d, [TP, H], f32, "c_b2b")
            Wout_sb = load_const(Wout_d, [H, OUT], f32, "c_wout")
            boutb_sb = load_const(boutb_d, [G, OUT], f32, "c_boutb")
            invb_sb = load_const(invb_d, [TP, G], f32, "c_invb")
            iota64_sb = load_const(iota64_d, [TP, G], f32, "c_iota64")
            batchf_sb = load_const(batchf_d, [TP, T], f32, "c_batchf")

            if resfold:
                xrow_sb = load_const(xrow_d, [TP, T * IN], bf16, "c_xrow")
                ident2_sb = load_const(ident2_d, [TP, TP], bf16, "c_ident2")
                # layer-1 bf16 output rows, kept resident for the L2 residual
                h1row_all = const.tile([TP, T * H], bf16, tag="c_h1rall")
                xT2_sb = h1T2_sb = None
            else:
                xT2_sb = const.tile([IN, SP], f32, tag="c_xt2")
                nc.sync.dma_start(xT2_sb[:], xT_d[:, :])
                nc.scalar.activation(xT2_sb[:], xT2_sb[:], Act.Copy,
                                     scale=2.0)
                h1T2_sb = const.tile([H, SP], f32, tag="c_h1t2")

            # DRAM bounce buffers for collectives (two tile-aligned chunks
            # so the first AllGather overlaps the tail of layer 1)
            h1_bounce_a = dram.tile([RA, H], bf16)
            h1_bounce_b = dram.tile([RB, H], bf16)
            h1A = dram.tile([C * RA, H], bf16)
            h1B = dram.tile([C * RB, H], bf16)
            pool_in = dram.tile([H, G], f32)
            pool_out = dram.tile([H, G], f32)

            def emit_ag(bounce, full, rows):
                if phase >= 2 and not fakecoll:
                    nc.gpsimd.collective_compute(
                        "AllGather",
                        mybir.AluOpType.bypass,
                        ins=[bounce.opt()],
                        outs=[full.opt()],
                        replica_groups=[list(range(C))],
                    )
                else:
                    for c in range(C):
                        nc.sync.dma_start(full[c * rows:(c + 1) * rows, :],
                                          bounce[:, :])

            # Tile assigns SWDGE completion-sem lanes (DMASW0-7) round-robin
            # over Pool DMA instructions in scheduled order; each sem is
            # HW-locked to one queue. Rotating queue_num in the same global
            # order (mod nq, with 8 lanes) keeps lane<->queue consistent.
            gctr = [0]

            def split_gather(mtile, table, isb, chunks, elem):
                """Issue one dma_gather per SWDGE queue over disjoint
                contiguous chunk ranges of the same destination tile."""
                nsub = min(nq, chunks)
                bounds = [chunks * q // nsub for q in range(nsub + 1)]
                for q in range(nsub):
                    c0, c1 = bounds[q], bounds[q + 1]
                    if c1 == c0:
                        continue
                    nc.gpsimd.dma_gather(
                        mtile[:, c0:c1], table[:, :],
                        isb[:, c0 * 8:c1 * 8],
                        (c1 - c0) * 128, (c1 - c0) * 128, elem,
                        single_packet=False, queue_num=gctr[0] % nq)
                    gctr[0] += 1

            def gen_P(pt, d_sb, base, k):
                """pt[:, c, r] = (d_sb[:, base+c] == r) for c in [0, k)."""
                for c in range(k):
                    nc.vector.tensor_scalar(
                        pt[:, c, :], iota128_sb[:],
                        d_sb[:, base + c:base + c + 1], None,
                        op0=Alu.is_equal)

            # =============== Layer 1 =====================================
            for bi, (b0, b1) in enumerate(blocks):
                cb0, Mb = blk1[bi]
                if Mb > 0:
                    if bi == 0 and pre_b:
                        ib, db, mt = pre_b[0]
                    else:
                        ib = ipool.tile([TP, Mb * 8], i16, tag="ib1")
                        nc.sync.dma_start(
                            ib[:], idx1_d[:, cb0 * 8:(cb0 + Mb) * 8])
                        db = ipool.tile([TP, Mb], f32, tag="db1")
                        nc.sync.dma_start(db[:], d1_d[:, cb0:cb0 + Mb])
                        mt = m1pool.tile([TP, Mb, 2 * IN], bf16, tag="m1")
                        nc.gpsimd.dma_gather(
                            mt[:], x_pair[:, :], ib[:], Mb * 128, Mb * 128,
                            2 * IN, single_packet=(Mb * 128 <= 1024))
                lb = 0
                for t in range(b0, b1):
                    Mt = int(M1[t])
                    k0 = int(K1[t, 0])
                    mpT = work.tile([IN + 1, TP], f32, tag="mpT")
                    nc.vector.memset(mpT[IN:IN + 1, :], 1.0)
                    if resfold:
                        pA = psum2.tile([IN, TP], f32, tag="aggr")
                        if Mt > 0:
                            Pt = ppool.tile([TP, Mt, TP], bf16, tag="p1")
                            gen_P(Pt, db, lb, Mt)
                        for cc in range(Mt):
                            sl = (mt[:, lb + cc, 0:IN] if cc < k0
                                  else mt[:, lb + cc, IN:2 * IN])
                            nc.tensor.matmul(pA[:], sl, Pt[:, cc, :],
                                             start=(cc == 0), stop=False)
                        nc.tensor.matmul(
                            pA[:], xrow_sb[:, t * IN:(t + 1) * IN],
                            ident2_sb[:], start=(Mt == 0), stop=True)
                        lb += Mt
                        nc.scalar.activation(mpT[0:IN, :], pA[:], Act.Copy)
                    elif Mt > 0:
                        Pt = ppool.tile([TP, Mt, TP], bf16, tag="p1")
                        gen_P(Pt, db, lb, Mt)
                        pA = psum2.tile([IN, TP], f32, tag="aggr")
                        for cc in range(Mt):
                            sl = (mt[:, lb + cc, 0:IN] if cc < k0
                                  else mt[:, lb + cc, IN:2 * IN])
                            nc.tensor.matmul(pA[:], sl, Pt[:, cc, :],
                                             start=(cc == 0),
                                             stop=(cc == Mt - 1))
                        nc.vector.tensor_tensor(
                            mpT[0:IN, :], pA[:],
                            xT2_sb[:, t * TP:(t + 1) * TP], op=Alu.add)
                        lb += Mt
                    else:
                        nc.vector.tensor_copy(
                            mpT[0:IN, :], xT2_sb[:, t * TP:(t + 1) * TP])

                    # h1 row-major (bf16) for the layer-2 gather table
                    pB = psum2.tile([TP, H], f32, tag="wmm")
                    nc.tensor.matmul(pB[:], mpT[:], W1e_sb[:],
                                     start=True, stop=True)
                    if resfold:
                        h1row = h1row_all[:, t * H:(t + 1) * H]
                    else:
                        h1row_t = work.tile([TP, H], bf16, tag="h1row")
                        h1row = h1row_t[:]
                    nc.scalar.activation(h1row, pB[:], Act.Relu)
                    w = min(TP, S - t * TP)
                    if t < TA:
                        nc.sync.dma_start(
                            h1_bounce_a[t * TP:t * TP + w, :], h1row[:w, :])
                    else:
                        r0 = (t - TA) * TP
                        nc.sync.dma_start(
                            h1_bounce_b[r0:r0 + w, :], h1row[:w, :])
                    if t == TA - 1:
                        emit_ag(h1_bounce_a, h1A, RA)

                    if not resfold:
                        # 2*relu(h1 pre)^T into persistent shard
                        pC = psum2.tile([H, TP], f32, tag="wmm2")
                        nc.tensor.matmul(pC[:], W1e_sb[:], mpT[:],
                                         start=True, stop=True)
                        nc.scalar.activation(
                            h1T2_sb[:, t * TP:(t + 1) * TP], pC[:],
                            Act.Relu, scale=2.0)

            # =============== AllGather h1 (chunk B) ======================
            emit_ag(h1_bounce_b, h1B, RB)

            # =============== Layer 2 + pooling ===========================
            pPool = psum1.tile([H, G], f32, tag="pool")
            for bi, (b0, b1) in enumerate(blocks):
                if phase < 3:
                    break
                cb0, KBlo, KBhi, per_tile = blk2[bi]
                if KBlo > 0:
                    ibl = ipool.tile([TP, KBlo * 8], i16, tag="ib2l")
                    nc.sync.dma_start(
                        ibl[:], idx2_d[:, cb0 * 8:(cb0 + KBlo) * 8])
                    mlo = mpool.tile([TP, KBlo, H], bf16, tag="m2lo")
                    split_gather(mlo, h1A, ibl, KBlo, H)
                if KBhi > 0:
                    ibh = ipool.tile([TP, KBhi * 8], i16, tag="ib2h")
                    nc.sync.dma_start(
                        ibh[:], idx2_d[:, (cb0 + KBlo) * 8:
                                       (cb0 + KBlo + KBhi) * 8])
                    mhi = mpool.tile([TP, KBhi, H], bf16, tag="m2hi")
                    split_gather(mhi, h1B, ibh, KBhi, H)
                db2 = ipool.tile([TP, KBlo + KBhi], f32, tag="db2")
                nc.sync.dma_start(db2[:], d2_d[:, cb0:cb0 + KBlo + KBhi])

                lo_off = 0
                hi_off = 0
                for ti, t in enumerate(range(b0, b1)):
                    klo, khi = per_tile[ti]
                    Mt = klo + khi
                    mpT2 = work.tile([H, TP], f32, tag="mpT2")
                    if resfold:
                        pD = psum2.tile([H, TP], f32, tag="aggr")
                        Plo = Phi = None
                        if klo > 0:
                            Plo = ppool.tile([TP, klo, TP], bf16, tag="p2lo")
                            gen_P(Plo, db2, lo_off, klo)
                        if khi > 0:
                            Phi = ppool.tile([TP, khi, TP], bf16, tag="p2hi")
                            gen_P(Phi, db2, KBlo + hi_off, khi)
                        for cc in range(Mt):
                            if cc < klo:
                                sl = mlo[:, lo_off + cc, :]
                                pp = Plo[:, cc, :]
                            else:
                                sl = mhi[:, hi_off + cc - klo, :]
                                pp = Phi[:, cc - klo, :]
                            nc.tensor.matmul(pD[:], sl, pp,
                                             start=(cc == 0), stop=False)
                        nc.tensor.matmul(
                            pD[:], h1row_all[:, t * H:(t + 1) * H],
                            ident2_sb[:], start=(Mt == 0), stop=True)
                        nc.scalar.activation(mpT2[:], pD[:], Act.Copy)
                        lo_off += klo
                        hi_off += khi
                    elif Mt > 0:
                        pD = psum2.tile([H, TP], f32, tag="aggr")
                        Plo = Phi = None
                        if klo > 0:
                            Plo = ppool.tile([TP, klo, TP], bf16, tag="p2lo")
                            gen_P(Plo, db2, lo_off, klo)
                        if khi > 0:
                            Phi = ppool.tile([TP, khi, TP], bf16, tag="p2hi")
                            gen_P(Phi, db2, KBlo + hi_off, khi)
                        for cc in range(Mt):
                            if cc < klo:
                                sl = mlo[:, lo_off + cc, :]
                                pp = Plo[:, cc, :]
                            else:
                                sl = mhi[:, hi_off + cc - klo, :]
                                pp = Phi[:, cc - klo, :]
                            nc.tensor.matmul(pD[:], sl, pp,
                                             start=(cc == 0),
                                             stop=(cc == Mt - 1))
                        nc.vector.tensor_tensor(
                            mpT2[:], pD[:],
                            h1T2_sb[:, t * TP:(t + 1) * TP], op=Alu.add)
                        lo_off += klo
                        hi_off += khi
                    else:
                        nc.vector.tensor_copy(
                            mpT2[:], h1T2_sb[:, t * TP:(t + 1) * TP])

                    pE = psum2.tile([TP, H], f32, tag="wmm")
                    nc.tensor.matmul(pE[:], mpT2[:], W2_sb[:],
                                     start=True, stop=True)
                    h2a = work.tile([TP, H], f32, tag="h2a")
                    nc.vector.tensor_tensor(h2a[:], pE[:], b2b_sb[:],
                                            op=Alu.add)
                    h2row = work.tile([TP, H], f32, tag="h2row")
                    nc.scalar.activation(h2row[:], h2a[:], Act.Relu)

                    P2 = ppool.tile([TP, G], f32, tag="p2")
                    nc.vector.tensor_scalar(
                        P2[:], iota64_sb[:], batchf_sb[:, t:t + 1], None,
                        op0=Alu.is_equal)
                    nc.tensor.matmul(pPool[:], h2row[:], P2[:],
                                     start=(t == 0), stop=(t == T - 1))

            # =============== finalize ====================================
            if phase < 3:
                outsb = work.tile([G, OUT], f32, tag="outsb")
                nc.vector.memset(outsb[:], 0.0)
                nc.sync.dma_start(out_d[:, :], outsb[:])
            else:
                poolsb = work.tile([H, G], f32, tag="poolsb")
                nc.vector.tensor_tensor(poolsb[:], pPool[:], invb_sb[:],
                                        op=Alu.mult)
                nc.sync.dma_start(pool_in[:, :], poolsb[:])
                if phase >= 4 and not fakecoll:
                    nc.gpsimd.collective_compute(
                        "AllReduce",
                        mybir.AluOpType.add,
                        ins=[pool_in.opt()],
                        outs=[pool_out.opt()],
                        replica_groups=[list(range(C))],
                    )
                else:
                    nc.sync.dma_start(pool_out[:, :], poolsb[:])
                arT = work.tile([H, G], f32, tag="arT")
                nc.sync.dma_start(arT[:], pool_out[:, :])
                pF = psum2.tile([G, OUT], f32, tag="wmm2")
                nc.tensor.matmul(pF[:], arT[:], Wout_sb[:],
                                 start=True, stop=True)
                outsb = work.tile([G, OUT], f32, tag="outsb")
                nc.vector.tensor_tensor(outsb[:], pF[:], boutb_sb[:],
                                        op=Alu.add)
                nc.sync.dma_start(out_d[:, :], outsb[:])

    nc.compile()
    return nc


def make_in_maps(plan, x, W1, b1, W2, b2, Wout, bout):
    x_pair = np.ascontiguousarray(
        np.asarray(x, np.float32).astype(BF16).reshape(NH, 2 * IN))
    W1e = np.concatenate([np.asarray(W1, np.float32),
                          np.asarray(b1, np.float32)[None, :]], axis=0)
    b2b = np.tile(np.asarray(b2, np.float32)[None, :], (TP, 1))
    boutb = np.tile(np.asarray(bout, np.float32)[None, :], (G, 1))
    invb = np.tile(plan.inv[None, :], (TP, 1)).astype(np.float32)
    iota128 = np.tile(np.arange(TP, dtype=np.float32)[None, :],
                      (TP, 1)).astype(BF16)
    iota64 = np.tile(np.arange(G, dtype=np.float32)[None, :],
                     (TP, 1)).astype(np.float32)
    ident2 = (2.0 * np.eye(TP, dtype=np.float32)).astype(BF16)

    in_maps = []
    for c in range(C):
        in_maps.append({
            "x_pair": x_pair,
            "xT": plan.xT[c],
            "xrow": plan.xrow[c],
            "ident2": ident2,
            "W1e": np.ascontiguousarray(W1e, np.float32),
            "W2": np.ascontiguousarray(np.asarray(W2, np.float32)),
            "b2b": np.ascontiguousarray(b2b, np.float32),
            "Wout": np.ascontiguousarray(np.asarray(Wout, np.float32)),
            "boutb": np.ascontiguousarray(boutb, np.float32),
            "invb": np.ascontiguousarray(invb, np.float32),
            "iota128": iota128,
            "iota64": np.ascontiguousarray(iota64, np.float32),
            "batchf": plan.batchf[c],
            "idx1": plan.idx1[c],
            "d1": plan.d1[c],
            "idx2": plan.idx2[c],
            "d2": plan.d2[c],
        })
    return in_maps


def kernel(x, edge_index, batch, W1, b1, W2, b2, Wout, bout):
    global LAST_EXEC_NS, LAST_RESULTS
    x = np.asarray(x, np.float32)
    edge_index = np.asarray(edge_index, np.int32)
    batch = np.asarray(batch, np.int32)

    plan = preprocess(x, edge_index, batch)
    in_maps = make_in_maps(plan, x, W1, b1, W2, b2, Wout, bout)
    nc = build_program(plan)

    from concourse import bass_utils
    trace = bool(int(os.environ.get("GNN_TRACE", "0")))
    res = bass_utils.run_bass_kernel_spmd(
        nc, in_maps, core_ids=list(range(C)), trace=trace)
    LAST_EXEC_NS = res.exec_time_ns
    LAST_RESULTS = res
    return np.asarray(res.results[0]["out"], np.float32)



# revision 6
# speedup vs baseline: 1.4591x; 1.4591x over previous
"""Trainium2 Bass kernel for a 2-layer GNN (message passing + MLP + global mean pool).

Reference computation (per graph batch):
    mp(h)[r] = 2*h[r] + sum_{e: row[e]==r} h[col[e]]      (self loop + residual fold)
    h1 = relu(mp(x) @ W1 + b1)
    h2 = relu(mp(h1) @ W2 + b2)
    out = segment_mean(h2, batch) @ Wout + bout

Strategy (8 NeuronCores):
  - Destination-shard nodes: core c owns rows [c*S, (c+1)*S), S = N/8.
  - Host (index-only preprocessing): bucket edges by dest shard, sort by dest
    row-tile (128 rows), split by source parity (L1, packed x pair table) /
    source chunk (L2, 4 tile-aligned AllGather chunks), pad chunk counts to
    the max across cores so all 8 cores run one program.
  - Device: dma_gather fetches bf16 source rows per edge, split across all 4
    SWDGE queues (desc-gen runs per-queue concurrently at ~8ns/desc; a
    single-queue gather serializes). Scatter-add runs on the TensorEngine as
    one-hot matmuls (P[k,r] = (dst[k]==r)) accumulating in PSUM per 128-row
    dest tile; P built with one DVE is_equal per chunk.
  - h1 AllGather in 4 tile-aligned chunks with Shared outputs, each launched
    as soon as its tiles finish layer 1, so all but the last hide under L1
    compute; L2 gathers are grouped by source chunk so they only wait on
    their own chunk's AllGather.
  - Global mean pool via one-hot matmul against graph ids; per-core partial
    [G, OUT] output AllReduced at the end.
"""

import os
import sys

for _p in ("/opt/trn_rl_repo", "/opt/pypackages"):
    if _p not in sys.path and os.path.isdir(_p):
        sys.path.append(_p)

import numpy as np
import ml_dtypes

BF16 = ml_dtypes.bfloat16

# Problem constants (nn_BasicGNN: N=50000 nodes, E=800000 edges).
N, E, IN, H, OUT, G = 50000, 800000, 64, 128, 10, 64
C = 8              # cores
S = N // C         # 6250 rows per shard
TP = 128           # rows per destination tile
T = (S + TP - 1) // TP   # 49 tiles per shard
SP = T * TP        # padded shard rows (6272)
NH = N // 2        # 25000: x pair-table rows
B = 4              # destination tiles per gather block
NQ = 4             # SWDGE queues (ucode max)
NCH = 4            # AllGather chunks / L2 source groups
CB = [0, 14, 27, 39, 49]          # chunk boundaries (tile indices)
RS = [cb * TP for cb in CB[:4]]   # chunk row starts
RSZ = [min(S, CB[k + 1] * TP) - RS[k] for k in range(NCH)]  # rows per chunk

PAD_DST = 255.0    # dest offset for padding messages (no row matches -> adds 0)

LAST_EXEC_NS = None
LAST_RESULTS = None


def _blocks():
    return [(b, min(b + B, T)) for b in range(0, T, B)]


def _wrap_idx(a):
    """int16 index array [K] (K%16==0) -> [128, K//16] in dma_gather layout:
    index i lives at [i % 16, i // 16], replicated for the 8 gpsimd cores."""
    K = a.shape[0]
    w = a.reshape(K // 16, 16).T.astype(np.int16)
    return np.tile(w, (8, 1))


def _wrap_dst(d):
    """dest-offset array [M*128] -> [128, M] f32; msg (c*128+k) -> [k, c]."""
    M = d.shape[0] // 128
    return d.reshape(M, 128).T.astype(np.float32)


class Plan:
    """Compile-time loop structure shared by all 8 cores + per-core tensors."""
    pass


def preprocess(x, edge_index, batch):
    """Index-only host preprocessing: edge bucketing/sorting + table packing."""
    plan = Plan()

    row = edge_index[0].astype(np.int64)
    col = edge_index[1].astype(np.int64)
    shard = row // S

    # counts per (core, tile, group) for both layers
    # L1 groups: source parity (pair table slicing); L2 groups: source chunk
    per_core = []
    cnt1 = np.zeros((C, T, 2), np.int64)
    cnt2 = np.zeros((C, T, NCH), np.int64)
    rs_arr = np.asarray(RS + [S], np.int64)
    for c in range(C):
        m = shard == c
        r = row[m] - c * S
        s = col[m]
        t = r // TP
        d = (r % TP).astype(np.float64)

        g1 = (s & 1).astype(np.int64)
        key1 = t * 2 + g1
        o1 = np.argsort(key1, kind="stable")
        cnt1[c] = np.bincount(key1, minlength=T * 2).reshape(T, 2)

        # L2: source split by local row chunk (tile-aligned AllGather chunks);
        # gather index into the rank-major chunk tables
        sr = s // S
        sl = s % S
        g2 = np.searchsorted(rs_arr, sl, side="right") - 1
        idx2v = sr * np.asarray(RSZ)[g2] + (sl - rs_arr[g2])
        key2 = t * NCH + g2
        o2 = np.argsort(key2, kind="stable")
        cnt2[c] = np.bincount(key2, minlength=T * NCH).reshape(T, NCH)

        per_core.append(
            dict(
                idx1=(s >> 1)[o1], dst1=d[o1],
                idx2=idx2v[o2], dst2=d[o2],
            )
        )

    # chunk counts (of 128 messages), maxed across cores -> single program
    K1 = np.maximum(-(-cnt1 // 128), 0).max(axis=0)   # [T, 2]
    K2 = np.maximum(-(-cnt2 // 128), 0).max(axis=0)   # [T, NCH]
    plan.K1 = K1
    plan.K2 = K2
    plan.M1 = K1.sum(axis=1)      # chunks per tile, layer 1
    plan.M2 = K2.sum(axis=1)      # chunks per tile, layer 2

    def starts_of(cnt, ng):
        starts = np.zeros((T, ng), np.int64)
        p = 0
        for t in range(T):
            for g in range(ng):
                starts[t, g] = p
                p += cnt[t, g]
        return starts

    def grab(idx, dst, starts, cnt, K, t, g):
        n = int(cnt[t, g])
        k = int(K[t, g])
        s0 = int(starts[t, g])
        ii = idx[s0:s0 + n]
        dd = dst[s0:s0 + n]
        pad = k * 128 - n
        if pad:
            ii = np.concatenate([ii, np.zeros(pad, np.int64)])
            dd = np.concatenate([dd, np.full(pad, PAD_DST)])
        return ii, dd

    def pack_l1(idx, dst, cnt):
        """L1 flat order: per tile [parity0 pad][parity1 pad]."""
        starts = starts_of(cnt, 2)
        idx_out, dst_out = [], []
        for t in range(T):
            for g in range(2):
                ii, dd = grab(idx, dst, starts, cnt, K1, t, g)
                idx_out.append(ii)
                dst_out.append(dd)
        return np.concatenate(idx_out), np.concatenate(dst_out)

    def pack_l2(idx, dst, cnt):
        """L2 flat order: per B-tile block [g0: t0..t3][g1: t0..t3]..."""
        starts = starts_of(cnt, NCH)
        idx_out, dst_out = [], []
        for b0, b1 in _blocks():
            for g in range(NCH):
                for t in range(b0, b1):
                    ii, dd = grab(idx, dst, starts, cnt, K2, t, g)
                    idx_out.append(ii)
                    dst_out.append(dd)
        return np.concatenate(idx_out), np.concatenate(dst_out)

    plan.idx1, plan.d1, plan.idx2, plan.d2 = [], [], [], []
    for c in range(C):
        pc = per_core[c]
        i1, dd1 = pack_l1(pc["idx1"], pc["dst1"], cnt1[c])
        i2, dd2 = pack_l2(pc["idx2"], pc["dst2"], cnt2[c])
        plan.idx1.append(_wrap_idx(i1))
        plan.d1.append(_wrap_dst(dd1))
        plan.idx2.append(_wrap_idx(i2))
        plan.d2.append(_wrap_dst(dd2))

    # per-core row-major x shard + batch ids
    plan.xrow = []
    plan.batchf = []
    for c in range(C):
        xs = np.zeros((SP, IN), np.float32)
        xs[:S] = x[c * S:(c + 1) * S]
        # row-major per-tile layout [128, T*IN]: [p, t*IN+f] = x[t*128+p, f]
        plan.xrow.append(np.ascontiguousarray(
            xs.reshape(T, TP, IN).transpose(1, 0, 2).reshape(TP, T * IN)
            .astype(BF16)))
        bf = np.full(SP, float(G), np.float32)
        bf[:S] = batch[c * S:(c + 1) * S].astype(np.float32)
        plan.batchf.append(np.ascontiguousarray(bf.reshape(T, TP).T))  # [128,T]

    # graph counts -> reciprocal (index-derived)
    cnts = np.bincount(batch.astype(np.int64), minlength=G).astype(np.float32)
    plan.inv = (1.0 / np.maximum(cnts, 1.0)).astype(np.float32)
    return plan


def build_program(plan):
    import concourse.bacc as bacc
    import concourse.tile as tile
    import concourse.mybir as mybir
    import concourse.tile_sem_assignment as _tsa

    # SWDGE completion sems are HW-locked to the queue that first bumps them,
    # but TileClockTick rotates DMASW lanes queue-blind. Pin lane = queue_num
    # for multi-queue SWDGE ops (same-queue ops serialize on the ring anyway,
    # so sharing one lane per queue adds no false dependencies).
    if not getattr(_tsa.TileClockTick, "_gnn_queue_lanes", False):
        _orig_assign = _tsa.TileClockTick._assign_tick

        def _assign(self, inst):
            qn = getattr(inst, "queue_num", None)
            if (qn is not None
                    and inst.engine == _tsa.mybir.EngineType.Pool
                    and isinstance(inst, _tsa.DMAInst)):
                self.next_sw_dma_idx = int(qn)
            return _orig_assign(self, inst)

        _tsa.TileClockTick._assign_tick = _assign
        _tsa.TileClockTick._gnn_queue_lanes = True

    dt = mybir.dt
    f32, bf16, i16 = dt.float32, dt.bfloat16, dt.int16
    Alu = mybir.AluOpType
    Act = mybir.ActivationFunctionType

    fakecoll = bool(int(os.environ.get("GNN_FAKECOLL", "0")))
    shared = bool(int(os.environ.get("GNN_SHARED", "1")))

    M1, M2, K1, K2 = plan.M1, plan.M2, plan.K1, plan.K2
    SM1 = int(M1.sum())
    SM2 = int(M2.sum())
    L1W = SM1 * 8
    L2W = SM2 * 8

    # per-tile chunk base offsets (L1 flat order)
    off1 = np.concatenate([[0], np.cumsum(M1)]).astype(np.int64)
    blocks = _blocks()
    # L1 block bookkeeping: (chunk base, chunks in block)
    blk1 = [(int(off1[b0]), int(off1[b1] - off1[b0])) for b0, b1 in blocks]
    # L2 block bookkeeping: (chunk base, per-group chunks, per-tile counts)
    blk2 = []
    cb = 0
    for b0, b1 in blocks:
        kg = [int(K2[b0:b1, g].sum()) for g in range(NCH)]
        blk2.append((cb, kg,
                     [[int(K2[t, g]) for g in range(NCH)]
                      for t in range(b0, b1)]))
        cb += sum(kg)

    nc = bacc.Bacc("TRN2", target_bir_lowering=False, debug=False,
                   num_devices=C, num_swdge_queues=NQ)

    # ---- I/O -------------------------------------------------------------
    x_pair = nc.dram_tensor("x_pair", [NH, 2 * IN], bf16, kind="ExternalInput")
    xrow_d = nc.dram_tensor("xrow", [TP, T * IN], bf16, kind="ExternalInput")
    ident2_d = nc.dram_tensor("ident2", [TP, TP], bf16, kind="ExternalInput")
    W1e_d = nc.dram_tensor("W1e", [IN + 1, H], f32, kind="ExternalInput")
    W2_d = nc.dram_tensor("W2", [H, H], f32, kind="ExternalInput")
    b2b_d = nc.dram_tensor("b2b", [TP, H], f32, kind="ExternalInput")
    Wout_d = nc.dram_tensor("Wout", [H, OUT], f32, kind="ExternalInput")
    boutb_d = nc.dram_tensor("boutb", [G, OUT], f32, kind="ExternalInput")
    invb_d = nc.dram_tensor("invb", [TP, G], f32, kind="ExternalInput")
    iota128_d = nc.dram_tensor("iota128", [TP, TP], bf16, kind="ExternalInput")
    iota64_d = nc.dram_tensor("iota64", [TP, G], f32, kind="ExternalInput")
    batchf_d = nc.dram_tensor("batchf", [TP, T], f32, kind="ExternalInput")
    idx1_d = nc.dram_tensor("idx1", [TP, L1W], i16, kind="ExternalInput")
    d1_d = nc.dram_tensor("d1", [TP, SM1], f32, kind="ExternalInput")
    idx2_d = nc.dram_tensor("idx2", [TP, L2W], i16, kind="ExternalInput")
    d2_d = nc.dram_tensor("d2", [TP, SM2], f32, kind="ExternalInput")
    out_d = nc.dram_tensor("out", [G, OUT], f32, kind="ExternalOutput")

    with tile.TileContext(nc) as tc:
        from contextlib import ExitStack
        with ExitStack() as ctx:
            const = ctx.enter_context(tc.tile_pool(name="const", bufs=1))
            work = ctx.enter_context(tc.tile_pool(name="work", bufs=3))
            mpool = ctx.enter_context(tc.tile_pool(name="mpool", bufs=2))
            m1pool = ctx.enter_context(tc.tile_pool(name="m1pool", bufs=2))
            ppool = ctx.enter_context(tc.tile_pool(name="ppool", bufs=2))
            ipool = ctx.enter_context(tc.tile_pool(name="ipool", bufs=2))
            psum2 = ctx.enter_context(
                tc.tile_pool(name="psum2", bufs=2, space="PSUM"))
            psum1 = ctx.enter_context(
                tc.tile_pool(name="psum1", bufs=1, space="PSUM"))
            dram = ctx.enter_context(
                tc.tile_pool(name="dram", bufs=1, space="DRAM"))

            # ---- constants / persistent SBUF ----------------------------
            def load_const(dram_t, shape, dtype, tag):
                t = const.tile(shape, dtype, tag=tag)
                nc.sync.dma_start(t[:], dram_t[:, :])
                return t

            W1e_sb = load_const(W1e_d, [IN + 1, H], f32, "c_w1e")
            iota128_sb = load_const(iota128_d, [TP, TP], bf16, "c_iota128")
            W2_sb = load_const(W2_d, [H, H], f32, "c_w2")
            b2b_sb = load_const(b2b_d, [TP, H], f32, "c_b2b")
            Wout_sb = load_const(Wout_d, [H, OUT], f32, "c_wout")
            boutb_sb = load_const(boutb_d, [G, OUT], f32, "c_boutb")
            invb_sb = load_const(invb_d, [TP, G], f32, "c_invb")
            iota64_sb = load_const(iota64_d, [TP, G], f32, "c_iota64")
            batchf_sb = load_const(batchf_d, [TP, T], f32, "c_batchf")
            xrow_sb = load_const(xrow_d, [TP, T * IN], bf16, "c_xrow")
            ident2_sb = load_const(ident2_d, [TP, TP], bf16, "c_ident2")
            # layer-1 bf16 output rows, kept resident for the L2 residual
            h1row_all = const.tile([TP, T * H], bf16, tag="c_h1rall")

            # DRAM bounce buffers for collectives (tile-aligned chunks so
            # each AllGather overlaps the next span of layer 1)
            agspace = "Shared" if shared else "Local"
            h1_bounce = [dram.tile([RSZ[k], H], bf16, name=f"h1bounce{k}")
                         for k in range(NCH)]
            h1_full = [dram.tile([C * RSZ[k], H], bf16, addr_space=agspace,
                                 name=f"h1full{k}")
                       for k in range(NCH)]
            pool_in = dram.tile([G, OUT], f32)
            pool_out = dram.tile([G, OUT], f32, addr_space=agspace)

            def emit_ag(k):
                if not fakecoll:
                    nc.gpsimd.collective_compute(
                        "AllGather",
                        mybir.AluOpType.bypass,
                        ins=[h1_bounce[k].opt()],
                        outs=[h1_full[k].opt()],
                        replica_groups=[list(range(C))],
                    )
                else:
                    for c in range(C):
                        nc.sync.dma_start(
                            h1_full[k][c * RSZ[k]:(c + 1) * RSZ[k], :],
                            h1_bounce[k][:, :])

            # Tile assigns SWDGE completion-sem lanes (DMASW0-7) round-robin
            # over Pool DMA instructions in scheduled order; each sem is
            # HW-locked to one queue. The _assign_tick pin above keeps
            # lane == queue_num.
            gctr = [0]

            def split_gather(mtile, table, isb, chunks, elem, parts):
                """Issue `parts` dma_gathers on rotating SWDGE queues over
                disjoint contiguous chunk ranges of one destination tile.
                Desc-gen runs per-queue concurrently (~8ns/desc/queue), so
                spreading a block's descriptors is what makes it fast."""
                nsub = min(parts, chunks)
                bounds = [chunks * q // nsub for q in range(nsub + 1)]
                for q in range(nsub):
                    c0, c1 = bounds[q], bounds[q + 1]
                    if c1 == c0:
                        continue
                    nc.gpsimd.dma_gather(
                        mtile[:, c0:c1], table[:, :],
                        isb[:, c0 * 8:c1 * 8],
                        (c1 - c0) * 128, (c1 - c0) * 128, elem,
                        single_packet=False, queue_num=gctr[0] % NQ)
                    gctr[0] += 1

            def gen_P(pt, pbase, d_sb, base, k):
                """pt[:, pbase+c, r] = (d_sb[:, base+c] == r) for c in [0, k)."""
                for c in range(k):
                    nc.vector.tensor_scalar(
                        pt[:, pbase + c, :], iota128_sb[:],
                        d_sb[:, base + c:base + c + 1], None,
                        op0=Alu.is_equal)

            # =============== Layer 1 =====================================
            for bi, (b0, b1) in enumerate(blocks):
                cb0, Mb = blk1[bi]
                if Mb > 0:
                    ib = ipool.tile([TP, Mb * 8], i16, tag="ib1")
                    nc.sync.dma_start(
                        ib[:], idx1_d[:, cb0 * 8:(cb0 + Mb) * 8])
                    db = ipool.tile([TP, Mb], f32, tag="db1")
                    nc.sync.dma_start(db[:], d1_d[:, cb0:cb0 + Mb])
                    mt = m1pool.tile([TP, Mb, 2 * IN], bf16, tag="m1")
                    split_gather(mt, x_pair, ib, Mb, 2 * IN, NQ)
                lb = 0
                for t in range(b0, b1):
                    Mt = int(M1[t])
                    k0 = int(K1[t, 0])
                    mpT = work.tile([IN + 1, TP], f32, tag="mpT")
                    nc.vector.memset(mpT[IN:IN + 1, :], 1.0)
                    pA = psum2.tile([IN, TP], f32, tag="aggr")
                    if Mt > 0:
                        Pt = ppool.tile([TP, Mt, TP], bf16, tag="p1")
                        gen_P(Pt, 0, db, lb, Mt)
                    for cc in range(Mt):
                        sl = (mt[:, lb + cc, 0:IN] if cc < k0
                              else mt[:, lb + cc, IN:2 * IN])
                        nc.tensor.matmul(pA[:], sl, Pt[:, cc, :],
                                         start=(cc == 0), stop=False)
                    nc.tensor.matmul(
                        pA[:], xrow_sb[:, t * IN:(t + 1) * IN],
                        ident2_sb[:], start=(Mt == 0), stop=True)
                    lb += Mt
                    nc.scalar.activation(mpT[0:IN, :], pA[:], Act.Copy)

                    # h1 row-major (bf16) for the layer-2 gather table
                    pB = psum2.tile([TP, H], f32, tag="wmm")
                    nc.tensor.matmul(pB[:], mpT[:], W1e_sb[:],
                                     start=True, stop=True)
                    h1row = h1row_all[:, t * H:(t + 1) * H]
                    nc.scalar.activation(h1row, pB[:], Act.Relu)
                    w = min(TP, S - t * TP)
                    k = next(kk for kk in range(NCH)
                             if CB[kk] <= t < CB[kk + 1])
                    r0 = t * TP - RS[k]
                    nc.sync.dma_start(
                        h1_bounce[k][r0:r0 + w, :], h1row[:w, :])
                    if t == CB[k + 1] - 1:
                        emit_ag(k)

            # =============== Layer 2 + pooling ===========================
            pPool = psum1.tile([H, G], f32, tag="pool")
            for bi, (b0, b1) in enumerate(blocks):
                cb0, kg, per_tile = blk2[bi]
                goff = np.concatenate([[0], np.cumsum(kg)]).astype(int)
                mg = []
                db2 = ipool.tile([TP, sum(kg)], f32, tag="db2")
                nc.sync.dma_start(db2[:], d2_d[:, cb0:cb0 + sum(kg)])
                for g in range(NCH):
                    if kg[g] == 0:
                        mg.append(None)
                        continue
                    ibg = ipool.tile([TP, kg[g] * 8], i16, tag=f"ib2_{g}")
                    nc.sync.dma_start(
                        ibg[:], idx2_d[:, (cb0 + goff[g]) * 8:
                                       (cb0 + goff[g + 1]) * 8])
                    mgt = mpool.tile([TP, kg[g], H], bf16, tag=f"m2_{g}")
                    split_gather(mgt, h1_full[g], ibg, kg[g], H, 1)
                    mg.append(mgt)

                run = [0] * NCH
                for ti, t in enumerate(range(b0, b1)):
                    ks = per_tile[ti]
                    Mt = sum(ks)
                    mpT2 = work.tile([H, TP], f32, tag="mpT2")
                    pD = psum2.tile([H, TP], f32, tag="aggr")
                    if Mt > 0:
                        Pt2 = ppool.tile([TP, Mt, TP], bf16, tag="p2")
                        lcc = 0
                        for g in range(NCH):
                            if ks[g]:
                                gen_P(Pt2, lcc, db2, goff[g] + run[g], ks[g])
                                lcc += ks[g]
                    first = True
                    lcc = 0
                    for g in range(NCH):
                        for cc in range(ks[g]):
                            nc.tensor.matmul(
                                pD[:], mg[g][:, run[g] + cc, :],
                                Pt2[:, lcc + cc, :],
                                start=first, stop=False)
                            first = False
                        lcc += ks[g]
                        run[g] += ks[g]
                    nc.tensor.matmul(
                        pD[:], h1row_all[:, t * H:(t + 1) * H],
                        ident2_sb[:], start=(Mt == 0), stop=True)
                    nc.scalar.activation(mpT2[:], pD[:], Act.Copy)

                    pE = psum2.tile([TP, H], f32, tag="wmm")
                    nc.tensor.matmul(pE[:], mpT2[:], W2_sb[:],
                                     start=True, stop=True)
                    h2a = work.tile([TP, H], f32, tag="h2a")
                    nc.vector.tensor_tensor(h2a[:], pE[:], b2b_sb[:],
                                            op=Alu.add)
                    h2row = work.tile([TP, H], f32, tag="h2row")
                    nc.scalar.activation(h2row[:], h2a[:], Act.Relu)

                    P2 = ppool.tile([TP, G], f32, tag="pgr")
                    nc.vector.tensor_scalar(
                        P2[:], iota64_sb[:], batchf_sb[:, t:t + 1], None,
                        op0=Alu.is_equal)
                    nc.tensor.matmul(pPool[:], h2row[:], P2[:],
                                     start=(t == 0), stop=(t == T - 1))

            # =============== finalize ====================================
            poolsb = work.tile([H, G], f32, tag="poolsb")
            nc.vector.tensor_tensor(poolsb[:], pPool[:], invb_sb[:],
                                    op=Alu.mult)
            pF = psum2.tile([G, OUT], f32, tag="wmm2")
            nc.tensor.matmul(pF[:], poolsb[:], Wout_sb[:],
                             start=True, stop=True)
            outp = work.tile([G, OUT], f32, tag="outp")
            nc.scalar.activation(outp[:], pF[:], Act.Copy)
            nc.sync.dma_start(pool_in[:, :], outp[:])
            if not fakecoll:
                nc.gpsimd.collective_compute(
                    "AllReduce",
                    mybir.AluOpType.add,
                    ins=[pool_in.opt()],
                    outs=[pool_out.opt()],
                    replica_groups=[list(range(C))],
                )
            else:
                nc.sync.dma_start(pool_out[:, :], outp[:])
            arT = work.tile([G, OUT], f32, tag="arT")
            nc.sync.dma_start(arT[:], pool_out[:, :])
            outsb = work.tile([G, OUT], f32, tag="outsb")
            nc.vector.tensor_tensor(outsb[:], arT[:], boutb_sb[:],
                                    op=Alu.add)
            nc.sync.dma_start(out_d[:, :], outsb[:])

    nc.compile()
    return nc


def make_in_maps(plan, x, W1, b1, W2, b2, Wout, bout):
    x_pair = np.ascontiguousarray(
        np.asarray(x, np.float32).astype(BF16).reshape(NH, 2 * IN))
    W1e = np.concatenate([np.asarray(W1, np.float32),
                          np.asarray(b1, np.float32)[None, :]], axis=0)
    b2b = np.tile(np.asarray(b2, np.float32)[None, :], (TP, 1))
    boutb = np.tile(np.asarray(bout, np.float32)[None, :], (G, 1))
    invb = np.tile(plan.inv[None, :], (TP, 1)).astype(np.float32)
    iota128 = np.tile(np.arange(TP, dtype=np.float32)[None, :],
                      (TP, 1)).astype(BF16)
    iota64 = np.tile(np.arange(G, dtype=np.float32)[None, :],
                     (TP, 1)).astype(np.float32)
    ident2 = (2.0 * np.eye(TP, dtype=np.float32)).astype(BF16)

    in_maps = []
    for c in range(C):
        in_maps.append({
            "x_pair": x_pair,
            "xrow": plan.xrow[c],
            "ident2": ident2,
            "W1e": np.ascontiguousarray(W1e, np.float32),
            "W2": np.ascontiguousarray(np.asarray(W2, np.float32)),
            "b2b": np.ascontiguousarray(b2b, np.float32),
            "Wout": np.ascontiguousarray(np.asarray(Wout, np.float32)),
            "boutb": np.ascontiguousarray(boutb, np.float32),
            "invb": np.ascontiguousarray(invb, np.float32),
            "iota128": iota128,
            "iota64": np.ascontiguousarray(iota64, np.float32),
            "batchf": plan.batchf[c],
            "idx1": plan.idx1[c],
            "d1": plan.d1[c],
            "idx2": plan.idx2[c],
            "d2": plan.d2[c],
        })
    return in_maps


def kernel(x, edge_index, batch, W1, b1, W2, b2, Wout, bout):
    global LAST_EXEC_NS, LAST_RESULTS
    x = np.asarray(x, np.float32)
    edge_index = np.asarray(edge_index, np.int32)
    batch = np.asarray(batch, np.int32)

    plan = preprocess(x, edge_index, batch)
    in_maps = make_in_maps(plan, x, W1, b1, W2, b2, Wout, bout)
    nc = build_program(plan)

    from concourse import bass_utils
    trace = bool(int(os.environ.get("GNN_TRACE", "0")))
    res = bass_utils.run_bass_kernel_spmd(
        nc, in_maps, core_ids=list(range(C)), trace=trace)
    LAST_EXEC_NS = res.exec_time_ns
    LAST_RESULTS = res
    return np.asarray(res.results[0]["out"], np.float32)


# revision 7
# speedup vs baseline: 1.6363x; 1.1215x over previous
"""Trainium2 Bass kernel for a 2-layer GNN (message passing + MLP + global mean pool).

Reference computation (per graph batch):
    mp(h)[r] = 2*h[r] + sum_{e: row[e]==r} h[col[e]]      (self loop + residual fold)
    h1 = relu(mp(x) @ W1 + b1)
    h2 = relu(mp(h1) @ W2 + b2)
    out = segment_mean(h2, batch) @ Wout + bout

Strategy (8 NeuronCores):
  - Destination-shard nodes: core c owns rows [c*S, (c+1)*S), S = N/8.
  - Host (index-only preprocessing): bucket edges by dest shard, sort by dest
    row-tile (128 rows), split by source parity (L1, packed x pair table) /
    source chunk (L2, 4 tile-aligned AllGather chunks), pad chunk counts to
    the max across cores so all 8 cores run one program.
  - Device: dma_gather fetches bf16 source rows per edge, split across all 4
    SWDGE queues (desc-gen runs per-queue concurrently at ~8ns/desc; a
    single-queue gather serializes). Scatter-add runs on the TensorEngine as
    one-hot matmuls (P[k,r] = (dst[k]==r)) accumulating in PSUM per 128-row
    dest tile; P built with one DVE is_equal per chunk.
  - h1 AllGather in 4 tile-aligned chunks with Shared outputs, each launched
    as soon as its tiles finish layer 1, so all but the last hide under L1
    compute; L2 gathers are grouped by source chunk so they only wait on
    their own chunk's AllGather.
  - Global mean pool via one-hot matmul against graph ids; per-core partial
    [G, OUT] output AllReduced at the end.
"""

import os
import sys

for _p in ("/opt/trn_rl_repo", "/opt/pypackages"):
    if _p not in sys.path and os.path.isdir(_p):
        sys.path.append(_p)

import numpy as np
import ml_dtypes

BF16 = ml_dtypes.bfloat16

# Problem constants (nn_BasicGNN: N=50000 nodes, E=800000 edges).
N, E, IN, H, OUT, G = 50000, 800000, 64, 128, 10, 64
C = 8              # cores
S = N // C         # 6250 rows per shard
TP = 128           # rows per destination tile
T = (S + TP - 1) // TP   # 49 tiles per shard
SP = T * TP        # padded shard rows (6272)
NH = N // 2        # 25000: x pair-table rows
B = 4              # destination tiles per gather block
NQ = 4             # SWDGE queues (ucode max)
NCH = 4            # AllGather chunks / L2 source groups
CB = [0, 14, 27, 39, 49]          # chunk boundaries (tile indices)
RS = [cb * TP for cb in CB[:4]]   # chunk row starts
RSZ = [min(S, CB[k + 1] * TP) - RS[k] for k in range(NCH)]  # rows per chunk

PAD_DST = 255.0    # dest offset for padding messages (no row matches -> adds 0)

LAST_EXEC_NS = None
LAST_RESULTS = None


def _blocks():
    return [(b, min(b + B, T)) for b in range(0, T, B)]


def _wrap_idx(a):
    """int16 index array [K] (K%16==0) -> [128, K//16] in dma_gather layout:
    index i lives at [i % 16, i // 16], replicated for the 8 gpsimd cores."""
    K = a.shape[0]
    w = a.reshape(K // 16, 16).T.astype(np.int16)
    return np.tile(w, (8, 1))


def _wrap_dst(d):
    """dest-offset array [M*128] -> [128, M] f32; msg (c*128+k) -> [k, c]."""
    M = d.shape[0] // 128
    return d.reshape(M, 128).T.astype(np.float32)


class Plan:
    """Compile-time loop structure shared by all 8 cores + per-core tensors."""
    pass


def preprocess(x, edge_index, batch):
    """Index-only host preprocessing: edge bucketing/sorting + table packing."""
    plan = Plan()

    row = edge_index[0].astype(np.int64)
    col = edge_index[1].astype(np.int64)
    shard = row // S

    # counts per (core, tile, group) for both layers
    # L1 groups: source parity (pair table slicing); L2 groups: source chunk
    per_core = []
    cnt1 = np.zeros((C, T, 2), np.int64)
    cnt2 = np.zeros((C, T, NCH), np.int64)
    rs_arr = np.asarray(RS + [S], np.int64)
    for c in range(C):
        m = shard == c
        r = row[m] - c * S
        s = col[m]
        t = r // TP
        d = (r % TP).astype(np.float64)

        g1 = (s & 1).astype(np.int64)
        key1 = t * 2 + g1
        o1 = np.argsort(key1, kind="stable")
        cnt1[c] = np.bincount(key1, minlength=T * 2).reshape(T, 2)

        # L2: source split by local row chunk (tile-aligned AllGather chunks);
        # gather index into the rank-major chunk tables
        sr = s // S
        sl = s % S
        g2 = np.searchsorted(rs_arr, sl, side="right") - 1
        idx2v = sr * np.asarray(RSZ)[g2] + (sl - rs_arr[g2])
        key2 = t * NCH + g2
        o2 = np.argsort(key2, kind="stable")
        cnt2[c] = np.bincount(key2, minlength=T * NCH).reshape(T, NCH)

        per_core.append(
            dict(
                idx1=(s >> 1)[o1], dst1=d[o1],
                idx2=idx2v[o2], dst2=d[o2],
            )
        )

    # chunk counts (of 128 messages), maxed across cores -> single program
    K1 = np.maximum(-(-cnt1 // 128), 0).max(axis=0)   # [T, 2]
    K2 = np.maximum(-(-cnt2 // 128), 0).max(axis=0)   # [T, NCH]
    plan.K1 = K1
    plan.K2 = K2
    plan.M1 = K1.sum(axis=1)      # chunks per tile, layer 1
    plan.M2 = K2.sum(axis=1)      # chunks per tile, layer 2

    def starts_of(cnt, ng):
        starts = np.zeros((T, ng), np.int64)
        p = 0
        for t in range(T):
            for g in range(ng):
                starts[t, g] = p
                p += cnt[t, g]
        return starts

    def grab(idx, dst, starts, cnt, K, t, g):
        n = int(cnt[t, g])
        k = int(K[t, g])
        s0 = int(starts[t, g])
        ii = idx[s0:s0 + n]
        dd = dst[s0:s0 + n]
        pad = k * 128 - n
        if pad:
            ii = np.concatenate([ii, np.zeros(pad, np.int64)])
            dd = np.concatenate([dd, np.full(pad, PAD_DST)])
        return ii, dd

    def pack_l1(idx, dst, cnt):
        """L1 flat order: per tile [parity0 pad][parity1 pad]."""
        starts = starts_of(cnt, 2)
        idx_out, dst_out = [], []
        for t in range(T):
            for g in range(2):
                ii, dd = grab(idx, dst, starts, cnt, K1, t, g)
                idx_out.append(ii)
                dst_out.append(dd)
        return np.concatenate(idx_out), np.concatenate(dst_out)

    def pack_l2(idx, dst, cnt):
        """L2 flat order: per B-tile block [g0: t0..t3][g1: t0..t3]..."""
        starts = starts_of(cnt, NCH)
        idx_out, dst_out = [], []
        for b0, b1 in _blocks():
            for g in range(NCH):
                for t in range(b0, b1):
                    ii, dd = grab(idx, dst, starts, cnt, K2, t, g)
                    idx_out.append(ii)
                    dst_out.append(dd)
        return np.concatenate(idx_out), np.concatenate(dst_out)

    plan.idx1, plan.d1, plan.idx2, plan.d2 = [], [], [], []
    for c in range(C):
        pc = per_core[c]
        i1, dd1 = pack_l1(pc["idx1"], pc["dst1"], cnt1[c])
        i2, dd2 = pack_l2(pc["idx2"], pc["dst2"], cnt2[c])
        plan.idx1.append(_wrap_idx(i1))
        plan.d1.append(_wrap_dst(dd1))
        plan.idx2.append(_wrap_idx(i2))
        plan.d2.append(_wrap_dst(dd2))

    # per-core row-major x shard + batch ids
    plan.xrow = []
    plan.batchf = []
    for c in range(C):
        xs = np.zeros((SP, IN), np.float32)
        xs[:S] = x[c * S:(c + 1) * S]
        # row-major per-tile layout [128, T*IN]: [p, t*IN+f] = x[t*128+p, f]
        plan.xrow.append(np.ascontiguousarray(
            xs.reshape(T, TP, IN).transpose(1, 0, 2).reshape(TP, T * IN)
            .astype(BF16)))
        bf = np.full(SP, float(G), np.float32)
        bf[:S] = batch[c * S:(c + 1) * S].astype(np.float32)
        plan.batchf.append(np.ascontiguousarray(bf.reshape(T, TP).T))  # [128,T]

    # graph counts -> reciprocal (index-derived)
    cnts = np.bincount(batch.astype(np.int64), minlength=G).astype(np.float32)
    plan.inv = (1.0 / np.maximum(cnts, 1.0)).astype(np.float32)
    return plan


def build_program(plan):
    import concourse.bacc as bacc
    import concourse.tile as tile
    import concourse.mybir as mybir
    import concourse.tile_sem_assignment as _tsa

    # SWDGE completion sems are HW-locked to the queue that first bumps them,
    # but TileClockTick rotates DMASW lanes queue-blind. Pin lane = queue_num
    # for multi-queue SWDGE ops (same-queue ops serialize on the ring anyway,
    # so sharing one lane per queue adds no false dependencies).
    if not getattr(_tsa.TileClockTick, "_gnn_queue_lanes", False):
        _orig_assign = _tsa.TileClockTick._assign_tick

        def _assign(self, inst):
            qn = getattr(inst, "queue_num", None)
            if (qn is not None
                    and inst.engine == _tsa.mybir.EngineType.Pool
                    and isinstance(inst, _tsa.DMAInst)):
                self.next_sw_dma_idx = int(qn)
            return _orig_assign(self, inst)

        _tsa.TileClockTick._assign_tick = _assign
        _tsa.TileClockTick._gnn_queue_lanes = True

    dt = mybir.dt
    f32, bf16, i16 = dt.float32, dt.bfloat16, dt.int16
    Alu = mybir.AluOpType
    Act = mybir.ActivationFunctionType

    fakecoll = bool(int(os.environ.get("GNN_FAKECOLL", "0")))
    shared = bool(int(os.environ.get("GNN_SHARED", "1")))

    M1, M2, K1, K2 = plan.M1, plan.M2, plan.K1, plan.K2
    SM1 = int(M1.sum())
    SM2 = int(M2.sum())
    L1W = SM1 * 8
    L2W = SM2 * 8

    # per-tile chunk base offsets (L1 flat order)
    off1 = np.concatenate([[0], np.cumsum(M1)]).astype(np.int64)
    blocks = _blocks()
    # L1 block bookkeeping: (chunk base, chunks in block)
    blk1 = [(int(off1[b0]), int(off1[b1] - off1[b0])) for b0, b1 in blocks]
    # L2 block bookkeeping: (chunk base, per-group chunks, per-tile counts)
    blk2 = []
    cb = 0
    for b0, b1 in blocks:
        kg = [int(K2[b0:b1, g].sum()) for g in range(NCH)]
        blk2.append((cb, kg,
                     [[int(K2[t, g]) for g in range(NCH)]
                      for t in range(b0, b1)]))
        cb += sum(kg)

    nc = bacc.Bacc("TRN2", target_bir_lowering=False, debug=False,
                   num_devices=C, num_swdge_queues=NQ)

    # ---- I/O -------------------------------------------------------------
    x_pair = nc.dram_tensor("x_pair", [NH, 2 * IN], bf16, kind="ExternalInput")
    xrow_d = nc.dram_tensor("xrow", [TP, T * IN], bf16, kind="ExternalInput")
    ident2_d = nc.dram_tensor("ident2", [TP, TP], bf16, kind="ExternalInput")
    W1e_d = nc.dram_tensor("W1e", [IN + 1, H], f32, kind="ExternalInput")
    W2_d = nc.dram_tensor("W2", [H, H], f32, kind="ExternalInput")
    b2b_d = nc.dram_tensor("b2b", [TP, H], f32, kind="ExternalInput")
    Wout_d = nc.dram_tensor("Wout", [H, OUT], f32, kind="ExternalInput")
    boutb_d = nc.dram_tensor("boutb", [G, OUT], f32, kind="ExternalInput")
    invb_d = nc.dram_tensor("invb", [TP, G], f32, kind="ExternalInput")
    iota128_d = nc.dram_tensor("iota128", [TP, TP], bf16, kind="ExternalInput")
    iota64_d = nc.dram_tensor("iota64", [TP, G], f32, kind="ExternalInput")
    batchf_d = nc.dram_tensor("batchf", [TP, T], f32, kind="ExternalInput")
    idx1_d = nc.dram_tensor("idx1", [TP, L1W], i16, kind="ExternalInput")
    d1_d = nc.dram_tensor("d1", [TP, SM1], f32, kind="ExternalInput")
    idx2_d = nc.dram_tensor("idx2", [TP, L2W], i16, kind="ExternalInput")
    d2_d = nc.dram_tensor("d2", [TP, SM2], f32, kind="ExternalInput")
    out_d = nc.dram_tensor("out", [G, OUT], f32, kind="ExternalOutput")

    with tile.TileContext(nc) as tc:
        from contextlib import ExitStack
        with ExitStack() as ctx:
            const = ctx.enter_context(tc.tile_pool(name="const", bufs=1))
            work = ctx.enter_context(tc.tile_pool(name="work", bufs=3))
            mpool = ctx.enter_context(tc.tile_pool(name="mpool", bufs=3))
            m1pool = ctx.enter_context(tc.tile_pool(name="m1pool", bufs=3))
            ppool = ctx.enter_context(tc.tile_pool(name="ppool", bufs=2))
            ipool = ctx.enter_context(tc.tile_pool(name="ipool", bufs=3))
            psum2 = ctx.enter_context(
                tc.tile_pool(name="psum2", bufs=2, space="PSUM"))
            psum1 = ctx.enter_context(
                tc.tile_pool(name="psum1", bufs=1, space="PSUM"))
            dram = ctx.enter_context(
                tc.tile_pool(name="dram", bufs=1, space="DRAM"))

            # ---- constants / persistent SBUF ----------------------------
            def load_const(dram_t, shape, dtype, tag):
                t = const.tile(shape, dtype, tag=tag)
                nc.sync.dma_start(t[:], dram_t[:, :])
                return t

            W1e_sb = load_const(W1e_d, [IN + 1, H], f32, "c_w1e")
            iota128_sb = load_const(iota128_d, [TP, TP], bf16, "c_iota128")
            W2_sb = load_const(W2_d, [H, H], f32, "c_w2")
            b2b_sb = load_const(b2b_d, [TP, H], f32, "c_b2b")
            Wout_sb = load_const(Wout_d, [H, OUT], f32, "c_wout")
            boutb_sb = load_const(boutb_d, [G, OUT], f32, "c_boutb")
            invb_sb = load_const(invb_d, [TP, G], f32, "c_invb")
            iota64_sb = load_const(iota64_d, [TP, G], f32, "c_iota64")
            batchf_sb = load_const(batchf_d, [TP, T], f32, "c_batchf")
            xrow_sb = load_const(xrow_d, [TP, T * IN], bf16, "c_xrow")
            ident2_sb = load_const(ident2_d, [TP, TP], bf16, "c_ident2")
            # layer-1 bf16 output rows, kept resident for the L2 residual
            h1row_all = const.tile([TP, T * H], bf16, tag="c_h1rall")

            # DRAM bounce buffers for collectives (tile-aligned chunks so
            # each AllGather overlaps the next span of layer 1)
            agspace = "Shared" if shared else "Local"
            h1_bounce = [dram.tile([RSZ[k], H], bf16, name=f"h1bounce{k}")
                         for k in range(NCH)]
            h1_full = [dram.tile([C * RSZ[k], H], bf16, addr_space=agspace,
                                 name=f"h1full{k}")
                       for k in range(NCH)]
            pool_in = dram.tile([G, OUT], f32)
            pool_out = dram.tile([G, OUT], f32, addr_space=agspace)

            def emit_ag(k):
                if not fakecoll:
                    nc.gpsimd.collective_compute(
                        "AllGather",
                        mybir.AluOpType.bypass,
                        ins=[h1_bounce[k].opt()],
                        outs=[h1_full[k].opt()],
                        replica_groups=[list(range(C))],
                    )
                else:
                    for c in range(C):
                        nc.sync.dma_start(
                            h1_full[k][c * RSZ[k]:(c + 1) * RSZ[k], :],
                            h1_bounce[k][:, :])

            # Tile assigns SWDGE completion-sem lanes (DMASW0-7) round-robin
            # over Pool DMA instructions in scheduled order; each sem is
            # HW-locked to one queue. The _assign_tick pin above keeps
            # lane == queue_num.
            gctr = [0]

            def split_gather(mtile, table, isb, chunks, elem, parts):
                """Issue `parts` dma_gathers on rotating SWDGE queues over
                disjoint contiguous chunk ranges of one destination tile.
                Desc-gen runs per-queue concurrently (~8ns/desc/queue), so
                spreading a block's descriptors is what makes it fast."""
                nsub = min(parts, chunks)
                bounds = [chunks * q // nsub for q in range(nsub + 1)]
                for q in range(nsub):
                    c0, c1 = bounds[q], bounds[q + 1]
                    if c1 == c0:
                        continue
                    nc.gpsimd.dma_gather(
                        mtile[:, c0:c1], table[:, :],
                        isb[:, c0 * 8:c1 * 8],
                        (c1 - c0) * 128, (c1 - c0) * 128, elem,
                        single_packet=False, queue_num=gctr[0] % NQ)
                    gctr[0] += 1

            def gen_P(pt, pbase, d_sb, base, k):
                """pt[:, pbase+c, r] = (d_sb[:, base+c] == r) for c in [0, k)."""
                for c in range(k):
                    nc.vector.tensor_scalar(
                        pt[:, pbase + c, :], iota128_sb[:],
                        d_sb[:, base + c:base + c + 1], None,
                        op0=Alu.is_equal)

            # =============== Layer 1 =====================================
            for bi, (b0, b1) in enumerate(blocks):
                cb0, Mb = blk1[bi]
                if Mb > 0:
                    ib = ipool.tile([TP, Mb * 8], i16, tag="ib1")
                    nc.sync.dma_start(
                        ib[:], idx1_d[:, cb0 * 8:(cb0 + Mb) * 8])
                    db = ipool.tile([TP, Mb], f32, tag="db1")
                    nc.sync.dma_start(db[:], d1_d[:, cb0:cb0 + Mb])
                    mt = m1pool.tile([TP, Mb, 2 * IN], bf16, tag="m1")
                    split_gather(mt, x_pair, ib, Mb, 2 * IN, NQ)
                lb = 0
                for t in range(b0, b1):
                    Mt = int(M1[t])
                    k0 = int(K1[t, 0])
                    mpT = work.tile([IN + 1, TP], f32, tag="mpT")
                    nc.vector.memset(mpT[IN:IN + 1, :], 1.0)
                    pA = psum2.tile([IN, TP], f32, tag="aggr")
                    if Mt > 0:
                        Pt = ppool.tile([TP, Mt, TP], bf16, tag="p1")
                        gen_P(Pt, 0, db, lb, Mt)
                    for cc in range(Mt):
                        sl = (mt[:, lb + cc, 0:IN] if cc < k0
                              else mt[:, lb + cc, IN:2 * IN])
                        nc.tensor.matmul(pA[:], sl, Pt[:, cc, :],
                                         start=(cc == 0), stop=False)
                    nc.tensor.matmul(
                        pA[:], xrow_sb[:, t * IN:(t + 1) * IN],
                        ident2_sb[:], start=(Mt == 0), stop=True)
                    lb += Mt
                    nc.scalar.activation(mpT[0:IN, :], pA[:], Act.Copy)

                    # h1 row-major (bf16) for the layer-2 gather table
                    pB = psum2.tile([TP, H], f32, tag="wmm")
                    nc.tensor.matmul(pB[:], mpT[:], W1e_sb[:],
                                     start=True, stop=True)
                    h1row = h1row_all[:, t * H:(t + 1) * H]
                    nc.scalar.activation(h1row, pB[:], Act.Relu)
                    w = min(TP, S - t * TP)
                    k = next(kk for kk in range(NCH)
                             if CB[kk] <= t < CB[kk + 1])
                    r0 = t * TP - RS[k]
                    nc.sync.dma_start(
                        h1_bounce[k][r0:r0 + w, :], h1row[:w, :])
                    if t == CB[k + 1] - 1:
                        emit_ag(k)

            # =============== Layer 2 + pooling ===========================
            pPool = psum1.tile([H, G], f32, tag="pool")
            for bi, (b0, b1) in enumerate(blocks):
                cb0, kg, per_tile = blk2[bi]
                goff = np.concatenate([[0], np.cumsum(kg)]).astype(int)
                mg = []
                db2 = ipool.tile([TP, sum(kg)], f32, tag="db2")
                nc.sync.dma_start(db2[:], d2_d[:, cb0:cb0 + sum(kg)])
                for g in range(NCH):
                    if kg[g] == 0:
                        mg.append(None)
                        continue
                    ibg = ipool.tile([TP, kg[g] * 8], i16, tag=f"ib2_{g}")
                    nc.sync.dma_start(
                        ibg[:], idx2_d[:, (cb0 + goff[g]) * 8:
                                       (cb0 + goff[g + 1]) * 8])
                    mgt = mpool.tile([TP, kg[g], H], bf16, tag=f"m2_{g}")
                    split_gather(mgt, h1_full[g], ibg, kg[g], H, 1)
                    mg.append(mgt)

                run = [0] * NCH
                for ti, t in enumerate(range(b0, b1)):
                    ks = per_tile[ti]
                    Mt = sum(ks)
                    mpT2 = work.tile([H, TP], f32, tag="mpT2")
                    pD = psum2.tile([H, TP], f32, tag="aggr")
                    if Mt > 0:
                        Pt2 = ppool.tile([TP, Mt, TP], bf16, tag="p2")
                        lcc = 0
                        for g in range(NCH):
                            if ks[g]:
                                gen_P(Pt2, lcc, db2, goff[g] + run[g], ks[g])
                                lcc += ks[g]
                    first = True
                    lcc = 0
                    for g in range(NCH):
                        for cc in range(ks[g]):
                            nc.tensor.matmul(
                                pD[:], mg[g][:, run[g] + cc, :],
                                Pt2[:, lcc + cc, :],
                                start=first, stop=False)
                            first = False
                        lcc += ks[g]
                        run[g] += ks[g]
                    nc.tensor.matmul(
                        pD[:], h1row_all[:, t * H:(t + 1) * H],
                        ident2_sb[:], start=(Mt == 0), stop=True)
                    nc.scalar.activation(mpT2[:], pD[:], Act.Copy)

                    pE = psum2.tile([TP, H], f32, tag="wmm")
                    nc.tensor.matmul(pE[:], mpT2[:], W2_sb[:],
                                     start=True, stop=True)
                    h2a = work.tile([TP, H], f32, tag="h2a")
                    nc.vector.tensor_tensor(h2a[:], pE[:], b2b_sb[:],
                                            op=Alu.add)
                    h2row = work.tile([TP, H], f32, tag="h2row")
                    nc.scalar.activation(h2row[:], h2a[:], Act.Relu)

                    P2 = ppool.tile([TP, G], f32, tag="pgr")
                    nc.vector.tensor_scalar(
                        P2[:], iota64_sb[:], batchf_sb[:, t:t + 1], None,
                        op0=Alu.is_equal)
                    nc.tensor.matmul(pPool[:], h2row[:], P2[:],
                                     start=(t == 0), stop=(t == T - 1))

            # =============== finalize ====================================
            poolsb = work.tile([H, G], f32, tag="poolsb")
            nc.vector.tensor_tensor(poolsb[:], pPool[:], invb_sb[:],
                                    op=Alu.mult)
            pF = psum2.tile([G, OUT], f32, tag="wmm2")
            nc.tensor.matmul(pF[:], poolsb[:], Wout_sb[:],
                             start=True, stop=True)
            outp = work.tile([G, OUT], f32, tag="outp")
            nc.scalar.activation(outp[:], pF[:], Act.Copy)
            nc.sync.dma_start(pool_in[:, :], outp[:])
            if not fakecoll:
                nc.gpsimd.collective_compute(
                    "AllReduce",
                    mybir.AluOpType.add,
                    ins=[pool_in.opt()],
                    outs=[pool_out.opt()],
                    replica_groups=[list(range(C))],
                )
            else:
                nc.sync.dma_start(pool_out[:, :], outp[:])
            arT = work.tile([G, OUT], f32, tag="arT")
            nc.sync.dma_start(arT[:], pool_out[:, :])
            outsb = work.tile([G, OUT], f32, tag="outsb")
            nc.vector.tensor_tensor(outsb[:], arT[:], boutb_sb[:],
                                    op=Alu.add)
            nc.sync.dma_start(out_d[:, :], outsb[:])

    nc.compile()
    return nc


def make_in_maps(plan, x, W1, b1, W2, b2, Wout, bout):
    x_pair = np.ascontiguousarray(
        np.asarray(x, np.float32).astype(BF16).reshape(NH, 2 * IN))
    W1e = np.concatenate([np.asarray(W1, np.float32),
                          np.asarray(b1, np.float32)[None, :]], axis=0)
    b2b = np.tile(np.asarray(b2, np.float32)[None, :], (TP, 1))
    boutb = np.tile(np.asarray(bout, np.float32)[None, :], (G, 1))
    invb = np.tile(plan.inv[None, :], (TP, 1)).astype(np.float32)
    iota128 = np.tile(np.arange(TP, dtype=np.float32)[None, :],
                      (TP, 1)).astype(BF16)
    iota64 = np.tile(np.arange(G, dtype=np.float32)[None, :],
                     (TP, 1)).astype(np.float32)
    ident2 = (2.0 * np.eye(TP, dtype=np.float32)).astype(BF16)

    in_maps = []
    for c in range(C):
        in_maps.append({
            "x_pair": x_pair,
            "xrow": plan.xrow[c],
            "ident2": ident2,
            "W1e": np.ascontiguousarray(W1e, np.float32),
            "W2": np.ascontiguousarray(np.asarray(W2, np.float32)),
            "b2b": np.ascontiguousarray(b2b, np.float32),
            "Wout": np.ascontiguousarray(np.asarray(Wout, np.float32)),
            "boutb": np.ascontiguousarray(boutb, np.float32),
            "invb": np.ascontiguousarray(invb, np.float32),
            "iota128": iota128,
            "iota64": np.ascontiguousarray(iota64, np.float32),
            "batchf": plan.batchf[c],
            "idx1": plan.idx1[c],
            "d1": plan.d1[c],
            "idx2": plan.idx2[c],
            "d2": plan.d2[c],
        })
    return in_maps


def kernel(x, edge_index, batch, W1, b1, W2, b2, Wout, bout):
    global LAST_EXEC_NS, LAST_RESULTS
    x = np.asarray(x, np.float32)
    edge_index = np.asarray(edge_index, np.int32)
    batch = np.asarray(batch, np.int32)

    plan = preprocess(x, edge_index, batch)
    in_maps = make_in_maps(plan, x, W1, b1, W2, b2, Wout, bout)
    nc = build_program(plan)

    from concourse import bass_utils
    trace = bool(int(os.environ.get("GNN_TRACE", "0")))
    res = bass_utils.run_bass_kernel_spmd(
        nc, in_maps, core_ids=list(range(C)), trace=trace)
    LAST_EXEC_NS = res.exec_time_ns
    LAST_RESULTS = res
    return np.asarray(res.results[0]["out"], np.float32)


# revision 11
# speedup vs baseline: 1.7595x; 1.0753x over previous
"""Trainium2 Bass kernel for a 2-layer GNN (message passing + MLP + global mean pool).

Reference computation (per graph batch):
    mp(h)[r] = 2*h[r] + sum_{e: row[e]==r} h[col[e]]      (self loop + residual fold)
    h1 = relu(mp(x) @ W1 + b1)
    h2 = relu(mp(h1) @ W2 + b2)
    out = segment_mean(h2, batch) @ Wout + bout

Strategy (8 NeuronCores):
  - Destination-shard nodes: core c owns rows [c*S, (c+1)*S), S = N/8.
  - Host (index-only preprocessing): bucket edges by dest shard, sort by dest
    row-tile (128 rows), split by source parity (L1, packed x pair table) /
    source chunk (L2, 4 tile-aligned AllGather chunks), pad chunk counts to
    the max across cores so all 8 cores run one program.
  - Device: dma_gather fetches bf16 source rows per edge, split across all 4
    SWDGE queues (desc-gen runs per-queue concurrently at ~8ns/desc; a
    single-queue gather serializes). Scatter-add runs on the TensorEngine as
    one-hot matmuls (P[k,r] = (dst[k]==r)) accumulating in PSUM per 128-row
    dest tile; P built with one DVE is_equal per chunk.
  - h1 AllGather in 4 tile-aligned chunks with Shared outputs, each launched
    as soon as its tiles finish layer 1, so all but the last hide under L1
    compute; L2 gathers are grouped by source chunk so they only wait on
    their own chunk's AllGather.
  - Global mean pool via one-hot matmul against graph ids; per-core partial
    [G, OUT] output AllReduced at the end.
"""

import os
import sys

for _p in ("/opt/trn_rl_repo", "/opt/pypackages"):
    if _p not in sys.path and os.path.isdir(_p):
        sys.path.append(_p)

import numpy as np
import ml_dtypes

BF16 = ml_dtypes.bfloat16

# Problem constants (nn_BasicGNN: N=50000 nodes, E=800000 edges).
N, E, IN, H, OUT, G = 50000, 800000, 64, 128, 10, 64
C = 8              # cores
S = N // C         # 6250 rows per shard
TP = 128           # rows per destination tile
T = (S + TP - 1) // TP   # 49 tiles per shard
SP = T * TP        # padded shard rows (6272)
NH = N // 2        # 25000: x pair-table rows
B = 4              # destination tiles per gather block
NQ = 4             # SWDGE queues (ucode max)
NCH = 4            # AllGather chunks / L2 source groups
CB = [0, 14, 27, 39, 49]          # chunk boundaries (tile indices)
RS = [cb * TP for cb in CB[:4]]   # chunk row starts
RSZ = [min(S, CB[k + 1] * TP) - RS[k] for k in range(NCH)]  # rows per chunk

PAD_DST = 255.0    # dest offset for padding messages (no row matches -> adds 0)

LAST_EXEC_NS = None
LAST_RESULTS = None


def _blocks():
    return [(b, min(b + B, T)) for b in range(0, T, B)]


def _wrap_idx(a):
    """int16 index array [K] (K%16==0) -> [128, K//16] in dma_gather layout:
    index i lives at [i % 16, i // 16], replicated for the 8 gpsimd cores."""
    K = a.shape[0]
    w = a.reshape(K // 16, 16).T.astype(np.int16)
    return np.tile(w, (8, 1))


def _wrap_dst(d):
    """dest-offset array [M*128] -> [128, M] f32; msg (c*128+k) -> [k, c]."""
    M = d.shape[0] // 128
    return d.reshape(M, 128).T.astype(np.float32)


class Plan:
    """Compile-time loop structure shared by all 8 cores + per-core tensors."""
    pass


def preprocess(x, edge_index, batch):
    """Index-only host preprocessing: edge bucketing/sorting + table packing."""
    plan = Plan()

    row = edge_index[0].astype(np.int64)
    col = edge_index[1].astype(np.int64)
    shard = row // S

    # counts per (core, tile, group) for both layers
    # L1 groups: source parity (pair table slicing); L2 groups: source chunk
    per_core = []
    cnt1 = np.zeros((C, T, 2), np.int64)
    cnt2 = np.zeros((C, T, NCH), np.int64)
    rs_arr = np.asarray(RS + [S], np.int64)
    for c in range(C):
        m = shard == c
        r = row[m] - c * S
        s = col[m]
        t = r // TP
        d = (r % TP).astype(np.float64)

        g1 = (s & 1).astype(np.int64)
        key1 = t * 2 + g1
        o1 = np.argsort(key1, kind="stable")
        cnt1[c] = np.bincount(key1, minlength=T * 2).reshape(T, 2)

        # L2: source split by local row chunk (tile-aligned AllGather chunks);
        # gather index into the rank-major chunk tables
        sr = s // S
        sl = s % S
        g2 = np.searchsorted(rs_arr, sl, side="right") - 1
        idx2v = sr * np.asarray(RSZ)[g2] + (sl - rs_arr[g2])
        key2 = t * NCH + g2
        o2 = np.argsort(key2, kind="stable")
        cnt2[c] = np.bincount(key2, minlength=T * NCH).reshape(T, NCH)

        per_core.append(
            dict(
                idx1=(s >> 1)[o1], dst1=d[o1],
                idx2=idx2v[o2], dst2=d[o2],
            )
        )

    # chunk counts (of 128 messages), maxed across cores -> single program
    K1 = np.maximum(-(-cnt1 // 128), 0).max(axis=0)   # [T, 2]
    K2 = np.maximum(-(-cnt2 // 128), 0).max(axis=0)   # [T, NCH]
    plan.K1 = K1
    plan.K2 = K2
    plan.M1 = K1.sum(axis=1)      # chunks per tile, layer 1
    plan.M2 = K2.sum(axis=1)      # chunks per tile, layer 2

    def starts_of(cnt, ng):
        starts = np.zeros((T, ng), np.int64)
        p = 0
        for t in range(T):
            for g in range(ng):
                starts[t, g] = p
                p += cnt[t, g]
        return starts

    def grab(idx, dst, starts, cnt, K, t, g):
        n = int(cnt[t, g])
        k = int(K[t, g])
        s0 = int(starts[t, g])
        ii = idx[s0:s0 + n]
        dd = dst[s0:s0 + n]
        pad = k * 128 - n
        if pad:
            ii = np.concatenate([ii, np.zeros(pad, np.int64)])
            dd = np.concatenate([dd, np.full(pad, PAD_DST)])
        return ii, dd

    def pack_l1(idx, dst, cnt):
        """L1 flat order: per tile [parity0 pad][parity1 pad]."""
        starts = starts_of(cnt, 2)
        idx_out, dst_out = [], []
        for t in range(T):
            for g in range(2):
                ii, dd = grab(idx, dst, starts, cnt, K1, t, g)
                idx_out.append(ii)
                dst_out.append(dd)
        return np.concatenate(idx_out), np.concatenate(dst_out)

    def pack_l2(idx, dst, cnt):
        """L2 flat order: per B-tile block [g0: t0..t3][g1: t0..t3]..."""
        starts = starts_of(cnt, NCH)
        idx_out, dst_out = [], []
        for b0, b1 in _blocks():
            for g in range(NCH):
                for t in range(b0, b1):
                    ii, dd = grab(idx, dst, starts, cnt, K2, t, g)
                    idx_out.append(ii)
                    dst_out.append(dd)
        return np.concatenate(idx_out), np.concatenate(dst_out)

    plan.idx1, plan.d1, plan.idx2, plan.d2 = [], [], [], []
    for c in range(C):
        pc = per_core[c]
        i1, dd1 = pack_l1(pc["idx1"], pc["dst1"], cnt1[c])
        i2, dd2 = pack_l2(pc["idx2"], pc["dst2"], cnt2[c])
        plan.idx1.append(_wrap_idx(i1))
        plan.d1.append(_wrap_dst(dd1))
        plan.idx2.append(_wrap_idx(i2))
        plan.d2.append(_wrap_dst(dd2))

    # per-core row-major x shard + batch ids
    plan.xrow = []
    plan.batchf = []
    for c in range(C):
        xs = np.zeros((SP, IN), np.float32)
        xs[:S] = x[c * S:(c + 1) * S]
        # row-major per-tile layout [128, T*IN]: [p, t*IN+f] = x[t*128+p, f]
        plan.xrow.append(np.ascontiguousarray(
            xs.reshape(T, TP, IN).transpose(1, 0, 2).reshape(TP, T * IN)
            .astype(BF16)))
        bf = np.full(SP, float(G), np.float32)
        bf[:S] = batch[c * S:(c + 1) * S].astype(np.float32)
        plan.batchf.append(np.ascontiguousarray(bf.reshape(T, TP).T))  # [128,T]

    # graph counts -> reciprocal (index-derived)
    cnts = np.bincount(batch.astype(np.int64), minlength=G).astype(np.float32)
    plan.inv = (1.0 / np.maximum(cnts, 1.0)).astype(np.float32)
    return plan


def build_program(plan):
    import concourse.bacc as bacc
    import concourse.tile as tile
    import concourse.mybir as mybir
    import concourse.tile_sem_assignment as _tsa

    # SWDGE completion sems are HW-locked to the queue that first bumps them,
    # but TileClockTick rotates DMASW lanes queue-blind. Pin lane = queue_num
    # for multi-queue SWDGE ops (same-queue ops serialize on the ring anyway,
    # so sharing one lane per queue adds no false dependencies).
    if not getattr(_tsa.TileClockTick, "_gnn_queue_lanes", False):
        _orig_assign = _tsa.TileClockTick._assign_tick

        def _assign(self, inst):
            qn = getattr(inst, "queue_num", None)
            if (qn is not None
                    and inst.engine == _tsa.mybir.EngineType.Pool
                    and isinstance(inst, _tsa.DMAInst)):
                self.next_sw_dma_idx = int(qn)
            return _orig_assign(self, inst)

        _tsa.TileClockTick._assign_tick = _assign
        _tsa.TileClockTick._gnn_queue_lanes = True

    dt = mybir.dt
    f32, bf16, i16 = dt.float32, dt.bfloat16, dt.int16
    Alu = mybir.AluOpType
    Act = mybir.ActivationFunctionType

    fakecoll = bool(int(os.environ.get("GNN_FAKECOLL", "0")))
    shared = bool(int(os.environ.get("GNN_SHARED", "1")))

    M1, M2, K1, K2 = plan.M1, plan.M2, plan.K1, plan.K2
    SM1 = int(M1.sum())
    SM2 = int(M2.sum())
    L1W = SM1 * 8
    L2W = SM2 * 8

    # per-tile chunk base offsets (L1 flat order)
    off1 = np.concatenate([[0], np.cumsum(M1)]).astype(np.int64)
    blocks = _blocks()
    # L1 block bookkeeping: (chunk base, chunks in block)
    blk1 = [(int(off1[b0]), int(off1[b1] - off1[b0])) for b0, b1 in blocks]
    # L2 block bookkeeping: (chunk base, per-group chunks, per-tile counts)
    blk2 = []
    cb = 0
    for b0, b1 in blocks:
        kg = [int(K2[b0:b1, g].sum()) for g in range(NCH)]
        blk2.append((cb, kg,
                     [[int(K2[t, g]) for g in range(NCH)]
                      for t in range(b0, b1)]))
        cb += sum(kg)

    nc = bacc.Bacc("TRN2", target_bir_lowering=False, debug=False,
                   num_devices=C, num_swdge_queues=NQ)

    # ---- I/O -------------------------------------------------------------
    x_pair = nc.dram_tensor("x_pair", [NH, 2 * IN], bf16, kind="ExternalInput")
    xrow_d = nc.dram_tensor("xrow", [TP, T * IN], bf16, kind="ExternalInput")
    ident2_d = nc.dram_tensor("ident2", [TP, TP], bf16, kind="ExternalInput")
    W1e_d = nc.dram_tensor("W1e", [IN + 1, H], f32, kind="ExternalInput")
    W2_d = nc.dram_tensor("W2", [H, H], f32, kind="ExternalInput")
    b2b_d = nc.dram_tensor("b2b", [TP, H], f32, kind="ExternalInput")
    Wout_d = nc.dram_tensor("Wout", [H, OUT], f32, kind="ExternalInput")
    boutb_d = nc.dram_tensor("boutb", [G, OUT], f32, kind="ExternalInput")
    invb_d = nc.dram_tensor("invb", [TP, G], f32, kind="ExternalInput")
    iota128_d = nc.dram_tensor("iota128", [TP, TP], bf16, kind="ExternalInput")
    iota64_d = nc.dram_tensor("iota64", [TP, G], f32, kind="ExternalInput")
    batchf_d = nc.dram_tensor("batchf", [TP, T], f32, kind="ExternalInput")
    idx1_d = nc.dram_tensor("idx1", [TP, L1W], i16, kind="ExternalInput")
    d1_d = nc.dram_tensor("d1", [TP, SM1], f32, kind="ExternalInput")
    idx2_d = nc.dram_tensor("idx2", [TP, L2W], i16, kind="ExternalInput")
    d2_d = nc.dram_tensor("d2", [TP, SM2], f32, kind="ExternalInput")
    out_d = nc.dram_tensor("out", [G, OUT], f32, kind="ExternalOutput")

    with tile.TileContext(nc) as tc:
        from contextlib import ExitStack
        with ExitStack() as ctx:
            const = ctx.enter_context(tc.tile_pool(name="const", bufs=1))
            work = ctx.enter_context(tc.tile_pool(name="work", bufs=3))
            mpool = ctx.enter_context(tc.tile_pool(name="mpool", bufs=3))
            m1pool = ctx.enter_context(tc.tile_pool(name="m1pool", bufs=3))
            ppool = ctx.enter_context(tc.tile_pool(name="ppool", bufs=2))
            psum2 = ctx.enter_context(
                tc.tile_pool(name="psum2", bufs=2, space="PSUM"))
            psum1 = ctx.enter_context(
                tc.tile_pool(name="psum1", bufs=1, space="PSUM"))
            dram = ctx.enter_context(
                tc.tile_pool(name="dram", bufs=1, space="DRAM"))

            # ---- constants / persistent SBUF ----------------------------
            def load_const(dram_t, shape, dtype, tag):
                t = const.tile(shape, dtype, tag=tag)
                nc.sync.dma_start(t[:], dram_t[:, :])
                return t

            W1e_sb = load_const(W1e_d, [IN + 1, H], f32, "c_w1e")
            iota128_sb = load_const(iota128_d, [TP, TP], bf16, "c_iota128")
            W2_sb = load_const(W2_d, [H, H], f32, "c_w2")
            b2b_sb = load_const(b2b_d, [TP, H], f32, "c_b2b")
            Wout_sb = load_const(Wout_d, [H, OUT], f32, "c_wout")
            boutb_sb = load_const(boutb_d, [G, OUT], f32, "c_boutb")
            invb_sb = load_const(invb_d, [TP, G], f32, "c_invb")
            iota64_sb = load_const(iota64_d, [TP, G], f32, "c_iota64")
            batchf_sb = load_const(batchf_d, [TP, T], f32, "c_batchf")
            xrow_sb = load_const(xrow_d, [TP, T * IN], bf16, "c_xrow")
            idx1_sb = load_const(idx1_d, [TP, L1W], i16, "c_idx1")
            d1_sb = load_const(d1_d, [TP, SM1], f32, "c_d1")
            idx2_sb = load_const(idx2_d, [TP, L2W], i16, "c_idx2")
            d2_sb = load_const(d2_d, [TP, SM2], f32, "c_d2")
            ident2_sb = load_const(ident2_d, [TP, TP], bf16, "c_ident2")
            # layer-1 bf16 output rows, kept resident for the L2 residual
            h1row_all = const.tile([TP, T * H], bf16, tag="c_h1rall")

            # DRAM bounce buffers for collectives (tile-aligned chunks so
            # each AllGather overlaps the next span of layer 1)
            agspace = "Shared" if shared else "Local"
            h1_bounce = [dram.tile([RSZ[k], H], bf16, name=f"h1bounce{k}")
                         for k in range(NCH)]
            h1_full = [dram.tile([C * RSZ[k], H], bf16, addr_space=agspace,
                                 name=f"h1full{k}")
                       for k in range(NCH)]
            pool_in = dram.tile([G, OUT], f32)
            pool_out = dram.tile([G, OUT], f32, addr_space=agspace)

            def emit_ag(k):
                if not fakecoll:
                    nc.gpsimd.collective_compute(
                        "AllGather",
                        mybir.AluOpType.bypass,
                        ins=[h1_bounce[k].opt()],
                        outs=[h1_full[k].opt()],
                        replica_groups=[list(range(C))],
                    )
                else:
                    for c in range(C):
                        nc.sync.dma_start(
                            h1_full[k][c * RSZ[k]:(c + 1) * RSZ[k], :],
                            h1_bounce[k][:, :])

            # Tile assigns SWDGE completion-sem lanes (DMASW0-7) round-robin
            # over Pool DMA instructions in scheduled order; each sem is
            # HW-locked to one queue. The _assign_tick pin above keeps
            # lane == queue_num.
            gctr = [0]

            def split_gather(mtile, table, isb, ibase, chunks, elem, parts):
                """Issue `parts` dma_gathers on rotating SWDGE queues over
                disjoint contiguous chunk ranges of one destination tile.
                Desc-gen runs per-queue concurrently (~8ns/desc/queue), so
                spreading a block's descriptors is what makes it fast."""
                nsub = min(parts, chunks)
                bounds = [chunks * q // nsub for q in range(nsub + 1)]
                for q in range(nsub):
                    c0, c1 = bounds[q], bounds[q + 1]
                    if c1 == c0:
                        continue
                    nc.gpsimd.dma_gather(
                        mtile[:, c0:c1], table[:, :],
                        isb[:, (ibase + c0) * 8:(ibase + c1) * 8],
                        (c1 - c0) * 128, (c1 - c0) * 128, elem,
                        single_packet=False, queue_num=gctr[0] % NQ)
                    gctr[0] += 1

            def gen_P(pt, pbase, d_sb, base, k):
                """pt[:, pbase+c, r] = (d_sb[:, base+c] == r) for c in [0, k)."""
                for c in range(k):
                    nc.vector.tensor_scalar(
                        pt[:, pbase + c, :], iota128_sb[:],
                        d_sb[:, base + c:base + c + 1], None,
                        op0=Alu.is_equal)

            # =============== Layer 1 =====================================
            # AG-k trigger waits on its chunk's bounce DMAs; emitting it 2
            # blocks later keeps that wait off the gather dispatch path.
            ag_at = {min((CB[k + 1] - 1) // B + 2, len(blocks) - 1): k
                     for k in range(NCH - 1)}
            for bi, (b0, b1) in enumerate(blocks):
                if bi in ag_at:
                    emit_ag(ag_at[bi])
                cb0, Mb = blk1[bi]
                if Mb > 0:
                    mt = m1pool.tile([TP, Mb, 2 * IN], bf16, tag="m1")
                    split_gather(mt, x_pair, idx1_sb, cb0, Mb, 2 * IN, NQ)
                lb = 0
                for t in range(b0, b1):
                    Mt = int(M1[t])
                    k0 = int(K1[t, 0])
                    mpT = work.tile([IN + 1, TP], f32, tag="mpT")
                    nc.vector.memset(mpT[IN:IN + 1, :], 1.0)
                    pA = psum2.tile([IN, TP], f32, tag="aggr")
                    if Mt > 0:
                        Pt = ppool.tile([TP, Mt, TP], bf16, tag="p1")
                        gen_P(Pt, 0, d1_sb, cb0 + lb, Mt)
                    for cc in range(Mt):
                        sl = (mt[:, lb + cc, 0:IN] if cc < k0
                              else mt[:, lb + cc, IN:2 * IN])
                        nc.tensor.matmul(pA[:], sl, Pt[:, cc, :],
                                         start=(cc == 0), stop=False)
                    nc.tensor.matmul(
                        pA[:], xrow_sb[:, t * IN:(t + 1) * IN],
                        ident2_sb[:], start=(Mt == 0), stop=True)
                    lb += Mt
                    nc.scalar.activation(mpT[0:IN, :], pA[:], Act.Copy)

                    # h1 row-major (bf16) for the layer-2 gather table
                    pB = psum2.tile([TP, H], f32, tag="wmm")
                    nc.tensor.matmul(pB[:], mpT[:], W1e_sb[:],
                                     start=True, stop=True)
                    h1row = h1row_all[:, t * H:(t + 1) * H]
                    nc.scalar.activation(h1row, pB[:], Act.Relu)
                    w = min(TP, S - t * TP)
                    k = next(kk for kk in range(NCH)
                             if CB[kk] <= t < CB[kk + 1])
                    r0 = t * TP - RS[k]
                    nc.sync.dma_start(
                        h1_bounce[k][r0:r0 + w, :], h1row[:w, :])
            emit_ag(NCH - 1)

            # =============== Layer 2 + pooling ===========================
            pPool = psum1.tile([H, G], f32, tag="pool")
            for bi, (b0, b1) in enumerate(blocks):
                cb0, kg, per_tile = blk2[bi]
                goff = np.concatenate([[0], np.cumsum(kg)]).astype(int)
                mg = []
                for g in range(NCH):
                    if kg[g] == 0:
                        mg.append(None)
                        continue
                    mgt = mpool.tile([TP, kg[g], H], bf16, tag=f"m2_{g}")
                    split_gather(mgt, h1_full[g], idx2_sb, cb0 + goff[g],
                                 kg[g], H, 1)
                    mg.append(mgt)

                run = [0] * NCH
                for ti, t in enumerate(range(b0, b1)):
                    ks = per_tile[ti]
                    Mt = sum(ks)
                    mpT2 = work.tile([H, TP], f32, tag="mpT2")
                    pD = psum2.tile([H, TP], f32, tag="aggr")
                    if Mt > 0:
                        Pt2 = ppool.tile([TP, Mt, TP], bf16, tag="p2")
                        lcc = 0
                        for g in range(NCH):
                            if ks[g]:
                                gen_P(Pt2, lcc, d2_sb,
                                      cb0 + goff[g] + run[g], ks[g])
                                lcc += ks[g]
                    first = True
                    lcc = 0
                    for g in range(NCH):
                        for cc in range(ks[g]):
                            nc.tensor.matmul(
                                pD[:], mg[g][:, run[g] + cc, :],
                                Pt2[:, lcc + cc, :],
                                start=first, stop=False)
                            first = False
                        lcc += ks[g]
                        run[g] += ks[g]
                    nc.tensor.matmul(
                        pD[:], h1row_all[:, t * H:(t + 1) * H],
                        ident2_sb[:], start=(Mt == 0), stop=True)
                    nc.scalar.activation(mpT2[:], pD[:], Act.Copy)

                    pE = psum2.tile([TP, H], f32, tag="wmm")
                    nc.tensor.matmul(pE[:], mpT2[:], W2_sb[:],
                                     start=True, stop=True)
                    h2a = work.tile([TP, H], f32, tag="h2a")
                    nc.vector.tensor_tensor(h2a[:], pE[:], b2b_sb[:],
                                            op=Alu.add)
                    h2row = work.tile([TP, H], f32, tag="h2row")
                    nc.scalar.activation(h2row[:], h2a[:], Act.Relu)

                    P2 = ppool.tile([TP, G], f32, tag="pgr")
                    nc.vector.tensor_scalar(
                        P2[:], iota64_sb[:], batchf_sb[:, t:t + 1], None,
                        op0=Alu.is_equal)
                    nc.tensor.matmul(pPool[:], h2row[:], P2[:],
                                     start=(t == 0), stop=(t == T - 1))

            # =============== finalize ====================================
            poolsb = work.tile([H, G], f32, tag="poolsb")
            nc.vector.tensor_tensor(poolsb[:], pPool[:], invb_sb[:],
                                    op=Alu.mult)
            pF = psum2.tile([G, OUT], f32, tag="wmm2")
            nc.tensor.matmul(pF[:], poolsb[:], Wout_sb[:],
                             start=True, stop=True)
            outp = work.tile([G, OUT], f32, tag="outp")
            nc.scalar.activation(outp[:], pF[:], Act.Copy)
            nc.sync.dma_start(pool_in[:, :], outp[:])
            if not fakecoll:
                nc.gpsimd.collective_compute(
                    "AllReduce",
                    mybir.AluOpType.add,
                    ins=[pool_in.opt()],
                    outs=[pool_out.opt()],
                    replica_groups=[list(range(C))],
                )
            else:
                nc.sync.dma_start(pool_out[:, :], outp[:])
            arT = work.tile([G, OUT], f32, tag="arT")
            nc.sync.dma_start(arT[:], pool_out[:, :])
            outsb = work.tile([G, OUT], f32, tag="outsb")
            nc.vector.tensor_tensor(outsb[:], arT[:], boutb_sb[:],
                                    op=Alu.add)
            nc.sync.dma_start(out_d[:, :], outsb[:])

    nc.compile()
    return nc


def make_in_maps(plan, x, W1, b1, W2, b2, Wout, bout):
    x_pair = np.ascontiguousarray(
        np.asarray(x, np.float32).astype(BF16).reshape(NH, 2 * IN))
    W1e = np.concatenate([np.asarray(W1, np.float32),
                          np.asarray(b1, np.float32)[None, :]], axis=0)
    b2b = np.tile(np.asarray(b2, np.float32)[None, :], (TP, 1))
    boutb = np.tile(np.asarray(bout, np.float32)[None, :], (G, 1))
    invb = np.tile(plan.inv[None, :], (TP, 1)).astype(np.float32)
    iota128 = np.tile(np.arange(TP, dtype=np.float32)[None, :],
                      (TP, 1)).astype(BF16)
    iota64 = np.tile(np.arange(G, dtype=np.float32)[None, :],
                     (TP, 1)).astype(np.float32)
    ident2 = (2.0 * np.eye(TP, dtype=np.float32)).astype(BF16)

    in_maps = []
    for c in range(C):
        in_maps.append({
            "x_pair": x_pair,
            "xrow": plan.xrow[c],
            "ident2": ident2,
            "W1e": np.ascontiguousarray(W1e, np.float32),
            "W2": np.ascontiguousarray(np.asarray(W2, np.float32)),
            "b2b": np.ascontiguousarray(b2b, np.float32),
            "Wout": np.ascontiguousarray(np.asarray(Wout, np.float32)),
            "boutb": np.ascontiguousarray(boutb, np.float32),
            "invb": np.ascontiguousarray(invb, np.float32),
            "iota128": iota128,
            "iota64": np.ascontiguousarray(iota64, np.float32),
            "batchf": plan.batchf[c],
            "idx1": plan.idx1[c],
            "d1": plan.d1[c],
            "idx2": plan.idx2[c],
            "d2": plan.d2[c],
        })
    return in_maps


def kernel(x, edge_index, batch, W1, b1, W2, b2, Wout, bout):
    global LAST_EXEC_NS, LAST_RESULTS
    x = np.asarray(x, np.float32)
    edge_index = np.asarray(edge_index, np.int32)
    batch = np.asarray(batch, np.int32)

    plan = preprocess(x, edge_index, batch)
    in_maps = make_in_maps(plan, x, W1, b1, W2, b2, Wout, bout)
    nc = build_program(plan)

    from concourse import bass_utils
    trace = bool(int(os.environ.get("GNN_TRACE", "0")))
    res = bass_utils.run_bass_kernel_spmd(
        nc, in_maps, core_ids=list(range(C)), trace=trace)
    LAST_EXEC_NS = res.exec_time_ns
    LAST_RESULTS = res
    return np.asarray(res.results[0]["out"], np.float32)
